# revision 1
# baseline (speedup 1.0000x reference)
"""Trainium2 Bass kernel for nn_CoreferenceResolver (coref UNet + pair decoder).

Sharding: core c handles batch b=c//2 and pair-half h=c%2 (496 of 992 pairs).
The gather/cosine/UNet stages are replicated on the two cores sharing a batch;
the extractor linears and group-bilinear decoder are sharded over pairs.
"""
import os
import sys

for _p in ("/opt/trn_rl_repo",):
    if os.path.isdir(_p) and _p not in sys.path:
        sys.path.insert(0, _p)

import numpy as np

import concourse.bass as bass
import concourse.tile as tile
from concourse import bacc, mybir
from concourse.bass_utils import run_bass_kernel_spmd

f32 = mybir.dt.float32
i16 = mybir.dt.int16
AF = mybir.ActivationFunctionType
OP = mybir.AluOpType
f32r = mybir.dt.float32r
bf16 = mybir.dt.bfloat16


def _f(ap):
    return ap.bitcast(mybir.dt.float32)


def _r(ap):
    """View an fp32 AP as float32r for full-rate PE streaming (N>=256)."""
    return ap.bitcast(f32r)

B, L, D, H = 4, 1024, 768, 12
NE, P = 32, 992
BLOCK = 64
G = D // BLOCK          # 12 groups
OUT_CH = 256
NCORES = 8
NH = P // 2             # 496 pairs per core
KD = D // 128           # 6 chunks of the D dim


def build_nc():
    nc = bacc.Bacc("TRN2", target_bir_lowering=False, debug=False, num_devices=NCORES)

    def inp(name, shape, dt=f32):
        return nc.dram_tensor(name, shape, dt, kind="ExternalInput")

    x_b      = inp("x_b", [L, D])
    ent_idx  = inp("ent_idx", [128, 2], i16)
    ent_mask = inp("ent_mask", [NE, 1])
    iota32   = inp("iota32", [NE, 1])
    ident    = inp("ident", [NE, NE])
    smat     = inp("smat", [128, 2], bf16)
    ones_r   = inp("ones_r", [1, 128], f32r)
    hi_f     = inp("hi_f", [1, NH])
    ti_f     = inp("ti_f", [1, NH])
    pair_idx = inp("pair_idx", [128, NH // 16], i16)

    enc1_w9  = inp("enc1_w9", [1, 9 * 64], f32r);        enc1_bp = inp("enc1_bp", [64, 1])
    enc2_w9  = inp("enc2_w9", [64, 9, 128], f32r);   enc2_bp = inp("enc2_bp", [128, 1])
    bott_w9  = inp("bott_w9", [128, 9, 256], f32r);  bott_bp = inp("bott_bp", [128, 2])
    ag2_wgp  = inp("ag2_wgp", [128, 2, 128], f32r)
    ag2_wxp  = inp("ag2_wxp", [128, 128], f32r)
    ag2_psip = inp("ag2_psip", [128, 1], f32r)
    dec2_w9  = inp("dec2_w9", [128, 3, 9, 128], f32r); dec2_bp = inp("dec2_bp", [128, 1])
    ag1_wgp  = inp("ag1_wgp", [128, 64], f32r)
    ag1_wxp  = inp("ag1_wxp", [64, 64], f32r)
    ag1_psip = inp("ag1_psip", [64, 1], f32r)
    dec1_w9a = inp("dec1_w9a", [128, 9, 64], f32r)
    dec1_w9b = inp("dec1_w9b", [64, 9, 64], f32r);   dec1_bp = inp("dec1_bp", [64, 1])
    fin_wp   = inp("fin_wp", [64, 256], f32r);       fin_bp  = inp("fin_bp", [128, 2])

    W1h = inp("W1h", [128, KD, D], f32r)   # head_w[:768] K-chunked
    W2h = inp("W2h", [128, 2, D], f32r)    # head_w[768:] K-chunked
    W1t = inp("W1t", [128, KD, D], f32r)
    W2t = inp("W2t", [128, 2, D], f32r)
    head_bp = inp("head_bp", [128, KD])
    tail_bp = inp("tail_bp", [128, KD])
    wdec = inp("wdec", [128, G, 128], f32r)   # rows 0:64 == rows 64:128 (host-duplicated)
    dec_bp = inp("dec_bp", [2, 1])

    y = nc.dram_tensor("y", [2, NH], f32, kind="ExternalOutput")

    from contextlib import ExitStack
    with tile.TileContext(nc) as tc, ExitStack() as _ctx:
        sbw = _ctx.enter_context(tc.tile_pool(name="sbw", bufs=1))   # persistent
        sbt = _ctx.enter_context(tc.tile_pool(name="sbt", bufs=3))   # rotating temps
        sws = _ctx.enter_context(tc.tile_pool(name="sws", bufs=4))   # streamed W1 chunks

        # ---------------- load persistent tensors ----------------
        def load(t, shape, dt=f32, name=None, early=False):
            tt = sbw.tile(shape, dt, tag=name or t.name)
            (nc.gpsimd if early else nc.sync).dma_start(tt[:], t[:])
            return tt

        t_eidx  = load(ent_idx, [128, 2], i16, "eidx", early=True)
        t_emask = load(ent_mask, [NE, 1], f32, "emask")
        t_iota  = load(iota32, [NE, 1], f32, "iota")
        t_ident = load(ident, [NE, NE], f32, "ident")
        t_smat  = load(smat, [128, 2], bf16, "smat")
        t_ones  = load(ones_r, [1, 128], f32r, "ones")
        t_hif   = load(hi_f, [1, NH], f32, "hif")
        t_tif   = load(ti_f, [1, NH], f32, "tif")
        pu_cm = tc.tile_pool(name="pu", bufs=3, space="PSUM")
        pu = pu_cm.__enter__()

        # ---------------- entity gather + normalize ----------------
        nrm   = sbw.tile([NE, D], f32, tag="nrm")
        nrmT  = sbw.tile([128, KD, NE], f32, tag="nrmT")
        normc = sbw.tile([NE, 1], f32, tag="normc")
        ent_raw = sbt.tile([128, 1, D], f32, tag="entraw")
        nc.gpsimd.dma_gather(ent_raw[:], x_b[:], t_eidx[:],
                             num_idxs=NE, num_idxs_reg=NE, elem_size=D)
        ent = ent_raw[0:NE, 0, :]
        sq = sbt.tile([NE, D], f32, tag="t")
        nc.vector.tensor_mul(sq[:], ent, ent)
        ss = sbt.tile([NE, 1], f32, tag="ss")
        nc.vector.reduce_sum(ss[:], sq[:], axis=mybir.AxisListType.X)
        nc.scalar.sqrt(normc[:], ss[:])
        nc.vector.tensor_single_scalar(normc[:], normc[:], 1e-13, op=OP.max)
        rinv = sbt.tile([NE, 1], f32, tag="rinv")
        nc.vector.reciprocal(rinv[:], normc[:])
        nc.vector.tensor_tensor(out=rinv[:], in0=rinv[:], in1=t_emask[:], op=OP.mult)
        nc.vector.tensor_scalar(out=nrm[:], in0=ent, scalar1=rinv[:],
                                scalar2=None, op0=OP.mult)
        for k in range(KD):
            p_t = pu.tile([128, NE], f32, tag="pu")
            nc.tensor.transpose(p_t[:], nrm[:, k * 128:(k + 1) * 128], t_ident[:])
            nc.vector.tensor_copy(_r(nrmT[:, k, :]), p_t[:])

        t_pidx  = load(pair_idx, [128, NH // 16], i16, "pidx")

        t_enc1w = load(enc1_w9, [1, 9 * 64], f32r, "enc1w")
        t_enc1b = load(enc1_bp, [64, 1], f32, "enc1b")
        t_enc2w = load(enc2_w9, [64, 9, 128], f32r, "enc2w")
        t_enc2b = load(enc2_bp, [128, 1], f32, "enc2b")
        t_bottw = load(bott_w9, [128, 9, 256], f32r, "bottw")
        t_bottb = load(bott_bp, [128, 2], f32, "bottb")
        t_ag2wg = load(ag2_wgp, [128, 2, 128], f32r, "ag2wg")
        t_ag2wx = load(ag2_wxp, [128, 128], f32r, "ag2wx")
        t_ag2ps = load(ag2_psip, [128, 1], f32r, "ag2ps")
        t_dec2w = load(dec2_w9, [128, 3, 9, 128], f32r, "dec2w")
        t_dec2b = load(dec2_bp, [128, 1], f32, "dec2b")
        t_ag1wg = load(ag1_wgp, [128, 64], f32r, "ag1wg")
        t_ag1wx = load(ag1_wxp, [64, 64], f32r, "ag1wx")
        t_ag1ps = load(ag1_psip, [64, 1], f32r, "ag1ps")
        t_dec1wa = load(dec1_w9a, [128, 9, 64], f32r, "dec1wa")
        t_dec1wb = load(dec1_w9b, [64, 9, 64], f32r, "dec1wb")
        t_dec1b = load(dec1_bp, [64, 1], f32, "dec1b")
        t_finw  = load(fin_wp, [64, 256], f32r, "finw")
        t_finb  = load(fin_bp, [128, 2], f32, "finb")
        t_w2h   = load(W2h, [128, 2, D], f32r, "w2h")
        t_w2t   = load(W2t, [128, 2, D], f32r, "w2t")
        t_hbp   = load(head_bp, [128, KD], f32, "hbp")
        t_tbp   = load(tail_bp, [128, KD], f32, "tbp")
        t_wdec  = load(wdec, [128, G, 128], f32r, "wdec")
        t_decb  = load(dec_bp, [2, 1], f32, "decb")

        # ---------------- persistent intermediates ----------------
        img0  = sbw.tile([1, 34 * 34], f32, tag="img0")
        c1p   = sbw.tile([64, 34 * 34], f32, tag="c1p")
        p1p   = sbw.tile([64, 18 * 18], f32, tag="p1p")
        c2p   = sbw.tile([128, 18 * 18], f32, tag="c2p")
        p2p   = sbw.tile([128, 10 * 10], f32, tag="p2p")
        u2p0  = sbw.tile([128, 18 * 18], f32, tag="u2p0")
        u2p1  = sbw.tile([128, 18 * 18], f32, tag="u2p1")
        att2p = sbw.tile([128, 18 * 18], f32, tag="att2p")
        d2s   = sbw.tile([128, 256], f32, tag="d2s")
        u1p   = sbw.tile([128, 34 * 34], f32, tag="u1p")
        att1p = sbw.tile([64, 34 * 34], f32, tag="att1p")
        d1s   = sbw.tile([64, 1024], f32, tag="d1s")
        amap0 = sbw.tile([128, 1024], f32, tag="amap0")
        amap1 = sbw.tile([128, 1024], f32, tag="amap1")

        ew1   = sbw.tile([NE, D], f32, tag="ew1")
        et1   = sbw.tile([NE, D], f32, tag="et1")
        ohhi  = sbw.tile([NE, NH], f32, tag="ohhi")
        ohti  = sbw.tile([NE, NH], f32, tag="ohti")
        htT0  = sbw.tile([128, NH], f32, tag="htT0")
        htT1  = sbw.tile([128, NH], f32, tag="htT1")
        hsT   = sbw.tile([128, KD, NH], f32, tag="hsT")
        tsT   = sbw.tile([128, KD, NH], f32, tag="tsT")

        # zero the padded borders once (rounded writes: the borders feed f32r matmuls)
        for t in (img0, c1p, p1p, c2p, p2p, u2p0, u2p1, att2p, u1p, att1p):
            nc.gpsimd.memset(t[:], 0.0)

        # ---------------- cosine matrix ----------------
        p_cos = pu.tile([NE, NE], f32, tag="pu")
        for k in range(KD):
            nc.tensor.matmul(p_cos[:], nrmT[:, k, :], nrmT[:, k, :],
                             start=(k == 0), stop=(k == KD - 1))
        s_cos = sbt.tile([NE, NE], f32, tag="scos")
        nc.vector.tensor_copy(_r(s_cos[:]), p_cos[:])

        # ---------------- UNet ----------------
        # enc1: one padded image (DMA issued from DVE right after the cos
        # copy - no cross-queue hop), then 9 taps x 2 halves of K=1 matmuls
        img0v = img0[:].rearrange("c (h w) -> c h w", h=34, w=34)
        nc.gpsimd.dma_start(_r(img0v[0:1, 1:33, 1:33]), _r(s_cos[:]))
        p_c1 = pu.tile([64, 1024], f32, tag="pu")
        for hh in range(2):
            n_mm = 0
            for tap in range(9):
                dy, dx = tap // 3, tap % 3
                rows = slice(dy + 16 * hh, dy + 16 * hh + 16)
                nc.tensor.matmul(p_c1[:, hh * 512:(hh + 1) * 512],
                                 t_enc1w[0:1, tap * 64:(tap + 1) * 64],
                                 _r(img0v[0:1, rows, dx:dx + 32]),
                                 start=(n_mm == 0), stop=(n_mm == 8))
                n_mm += 1
        c1pv = c1p[:].rearrange("c (h w) -> c h w", h=34, w=34)
        for hh in range(2):
            nc.scalar.activation(_r(c1pv[:, 1 + 16 * hh:17 + 16 * hh, 1:33]),
                                 p_c1[:, hh * 512:(hh + 1) * 512].rearrange(
                                     "c (h w) -> c h w", h=16, w=32),
                                 AF.Relu, bias=t_enc1b[:])

        # pool1 -> p1p interior [64, 16, 16]
        p1pv = p1p[:].rearrange("c (h w) -> c h w", h=18, w=18)
        tmp = sbt.tile([64, 16, 16], f32, tag="t")
        nc.vector.tensor_max(tmp[:], c1pv[:, 1:33:2, 1:33:2], c1pv[:, 1:33:2, 2:34:2])
        nc.vector.tensor_max(tmp[:], tmp[:], c1pv[:, 2:34:2, 1:33:2])
        nc.vector.tensor_max(_r(p1pv[:, 1:17, 1:17]), tmp[:], c1pv[:, 2:34:2, 2:34:2])

        # enc2: 9 shifted matmuls K=64
        p_c2 = pu.tile([128, 256], f32, tag="pu")
        for tap in range(9):
            dy, dx = tap // 3, tap % 3
            nc.tensor.matmul(p_c2[:], _r(t_enc2w[:, tap, :]),
                             _r(p1pv[:, dy:dy + 16, dx:dx + 16]),
                             start=(tap == 0), stop=(tap == 8))
        c2pv = c2p[:].rearrange("c (h w) -> c h w", h=18, w=18)
        nc.scalar.activation(_r(c2pv[:, 1:17, 1:17]),
                             p_c2[:].rearrange("c (h w) -> c h w", h=16, w=16),
                             AF.Relu, bias=t_enc2b[:])

        # pool2 -> p2p interior [128, 8, 8]
        p2pv = p2p[:].rearrange("c (h w) -> c h w", h=10, w=10)
        tmp2 = sbt.tile([128, 8, 8], f32, tag="t")
        nc.vector.tensor_max(tmp2[:], c2pv[:, 1:17:2, 1:17:2], c2pv[:, 1:17:2, 2:18:2])
        nc.vector.tensor_max(tmp2[:], tmp2[:], c2pv[:, 2:18:2, 1:17:2])
        nc.vector.tensor_max(_r(p2pv[:, 1:9, 1:9]), tmp2[:], c2pv[:, 2:18:2, 2:18:2])

        # bottleneck: 9 taps x 2 M-chunks, K=128
        c3 = []
        for mc in range(2):
            p_c3 = pu.tile([128, 64], f32, tag="pu")
            for tap in range(9):
                dy, dx = tap // 3, tap % 3
                nc.tensor.matmul(p_c3[:], t_bottw[:, tap, mc * 128:(mc + 1) * 128],
                                 _r(p2pv[:, dy:dy + 8, dx:dx + 8]),
                                 start=(tap == 0), stop=(tap == 8))
            c3s = sbt.tile([128, 8, 8], f32, tag=f"c3_{mc}")
            nc.scalar.activation(c3s[:], p_c3[:].rearrange("c (h w) -> c h w", h=8, w=8),
                                 AF.Relu, bias=t_bottb[:, mc:mc + 1])
            c3.append(c3s)

        # up2 -> u2p interior [128, 16, 16] x2 chunks
        for mc, (src, dst) in enumerate(((c3[0], u2p0), (c3[1], u2p1))):
            dv = dst[:].rearrange("c (h w) -> c h w", h=18, w=18)
            for i in range(2):
                for j in range(2):
                    nc.vector.tensor_copy(_r(dv[:, 1 + i:17:2, 1 + j:17:2]), src[:])

        u2p0v = u2p0[:].rearrange("c (h w) -> c h w", h=18, w=18)
        u2p1v = u2p1[:].rearrange("c (h w) -> c h w", h=18, w=18)

        # attention gate 2: relu(wg@u2 + wx@c2) -> psi -> sigmoid -> c2*a
        p_a2 = pu.tile([128, 256], f32, tag="pu")
        nc.tensor.matmul(p_a2[:], _r(t_ag2wg[:, 0, :]), _r(u2p0v[:, 1:17, 1:17]),
                         start=True, stop=False)
        nc.tensor.matmul(p_a2[:], _r(t_ag2wg[:, 1, :]), _r(u2p1v[:, 1:17, 1:17]),
                         start=False, stop=False)
        nc.tensor.matmul(p_a2[:], _r(t_ag2wx[:]), _r(c2pv[:, 1:17, 1:17]),
                         start=False, stop=True)
        r2 = sbt.tile([128, 256], f32, tag="t")
        nc.scalar.activation(_r(r2[:]), p_a2[:], AF.Relu)
        p_g2 = pu.tile([1, 256], f32, tag="pu")
        nc.tensor.matmul(p_g2[:], t_ag2ps[:], _r(r2[:]))
        a2 = sbt.tile([1, 256], f32, tag="a2")
        nc.scalar.activation(_r(a2[:]), p_g2[:], AF.Sigmoid)
        p_a2b = pu.tile([128, 256], f32, tag="pu")
        nc.tensor.matmul(p_a2b[:], t_ones[:], _r(a2[:]))
        att2pv = att2p[:].rearrange("c (h w) -> c h w", h=18, w=18)
        att2t = sbt.tile([128, 256], f32, tag="t")
        nc.vector.tensor_mul(att2t[:].rearrange("c (h w) -> c h w", h=16, w=16),
                             p_a2b[:].rearrange("c (h w) -> c h w", h=16, w=16),
                             c2pv[:, 1:17, 1:17])
        nc.vector.tensor_copy(_r(att2pv[:, 1:17, 1:17]),
                              att2t[:].rearrange("c (h w) -> c h w", h=16, w=16))

        # dec2: 9 taps x 3 K-chunks (u2p0, u2p1, att2p)
        p_d2 = pu.tile([128, 256], f32, tag="pu")
        srcs2 = (u2p0v, u2p1v, att2pv)
        n_mm = 0
        for tap in range(9):
            dy, dx = tap // 3, tap % 3
            for kc in range(3):
                nc.tensor.matmul(p_d2[:], _r(t_dec2w[:, kc, tap, :]),
                                 _r(srcs2[kc][:, dy:dy + 16, dx:dx + 16]),
                                 start=(n_mm == 0), stop=(n_mm == 26))
                n_mm += 1
        nc.scalar.activation(d2s[:], p_d2[:], AF.Relu, bias=t_dec2b[:])

        # up1 -> u1p interior [128, 32, 32]
        u1pv = u1p[:].rearrange("c (h w) -> c h w", h=34, w=34)
        d2v = d2s[:].rearrange("c (h w) -> c h w", h=16, w=16)
        for i in range(2):
            for j in range(2):
                nc.vector.tensor_copy(_r(u1pv[:, 1 + i:33:2, 1 + j:33:2]), d2v[:])

        # attention gate 1
        p_a1 = pu.tile([64, 1024], f32, tag="pu")
        for hh in range(2):
            rows = slice(1 + 16 * hh, 17 + 16 * hh)
            nc.tensor.matmul(p_a1[:, hh * 512:(hh + 1) * 512], _r(t_ag1wg[:]),
                             _r(u1pv[:, rows, 1:33]), start=True, stop=False)
            nc.tensor.matmul(p_a1[:, hh * 512:(hh + 1) * 512], _r(t_ag1wx[:]),
                             _r(c1pv[:, rows, 1:33]), start=False, stop=True)
        r1 = sbt.tile([64, 1024], f32, tag="t")
        nc.scalar.activation(_r(r1[:]), p_a1[:], AF.Relu)
        p_g1 = pu.tile([1, 1024], f32, tag="pu")
        for hh in range(2):
            nc.tensor.matmul(p_g1[:, hh * 512:(hh + 1) * 512], t_ag1ps[:],
                             _r(r1[:, hh * 512:(hh + 1) * 512]))
        a1 = sbt.tile([1, 1024], f32, tag="a1")
        nc.scalar.activation(_r(a1[:]), p_g1[:], AF.Sigmoid)
        p_a1b = pu.tile([64, 1024], f32, tag="pu")
        for hh in range(2):
            nc.tensor.matmul(p_a1b[:, hh * 512:(hh + 1) * 512], t_ones[:, :64],
                             _r(a1[:, hh * 512:(hh + 1) * 512]))
        att1pv = att1p[:].rearrange("c (h w) -> c h w", h=34, w=34)
        att1t = sbt.tile([64, 1024], f32, tag="t")
        nc.vector.tensor_mul(att1t[:].rearrange("c (h w) -> c h w", h=32, w=32),
                             p_a1b[:].rearrange("c (h w) -> c h w", h=32, w=32),
                             c1pv[:, 1:33, 1:33])
        nc.vector.tensor_copy(_r(att1pv[:, 1:33, 1:33]),
                              att1t[:].rearrange("c (h w) -> c h w", h=32, w=32))

        # dec1: 9 taps x (u1p K=128 + att1p K=64) x 2 N-halves
        p_d1 = pu.tile([64, 1024], f32, tag="pu")
        for hh in range(2):
            n_mm = 0
            for tap in range(9):
                dy, dx = tap // 3, tap % 3
                rows = slice(dy + 16 * hh, dy + 16 * hh + 16)
                nc.tensor.matmul(p_d1[:, hh * 512:(hh + 1) * 512],
                                 _r(t_dec1wa[:, tap, :]), _r(u1pv[:, rows, dx:dx + 32]),
                                 start=(n_mm == 0), stop=False)
                n_mm += 1
                nc.tensor.matmul(p_d1[:, hh * 512:(hh + 1) * 512],
                                 _r(t_dec1wb[:, tap, :]), _r(att1pv[:, rows, dx:dx + 32]),
                                 start=False, stop=(n_mm == 17))
                n_mm += 1
            nc.scalar.activation(_r(d1s[:, hh * 512:(hh + 1) * 512]),
                                 p_d1[:, hh * 512:(hh + 1) * 512],
                                 AF.Relu, bias=t_dec1b[:])

        # fin 1x1 conv -> amapT [256, 1024] in two chunks (with bias, no relu)
        for mc, dst in ((0, amap0), (1, amap1)):
            p_am = pu.tile([128, 1024], f32, tag="pu")
            for hh in range(2):
                nc.tensor.matmul(p_am[:, hh * 512:(hh + 1) * 512],
                                 _r(t_finw[:, mc * 128:(mc + 1) * 128]),
                                 _r(d1s[:, hh * 512:(hh + 1) * 512]))
            nc.scalar.activation(dst[:], p_am[:], AF.Identity, bias=t_finb[:, mc:mc + 1])

        # ---------------- extractor premultiplies ----------------
        # EW1 = ent @ head_w[:768]  (= maxnorm-scaled nrm @ W1), same for tail
        for (wsrc, dst) in ((W1h, ew1), (W1t, et1)):
            p_ew = pu.tile([NE, D], f32, tag="pu")
            for k in range(KD):
                wchunk = sws.tile([128, D], f32r, tag="wbig")
                nc.sync.dma_start(wchunk[:], wsrc[:, k, :])
                for n0, n1 in ((0, 512), (512, 768)):
                    nc.tensor.matmul(p_ew[:, n0:n1],
                                     _r(nrmT[:, k, :]), _r(wchunk[:, n0:n1]),
                                     start=(k == 0), stop=(k == KD - 1))
            nc.scalar.activation(_r(dst[:]), p_ew[:], AF.Copy, scale=normc[:])

        # one-hot selector matrices for hi / ti
        for (src, dst) in ((t_hif, ohhi), (t_tif, ohti)):
            bc = sbt.tile([NE, NH], f32, tag="t")
            nc.gpsimd.partition_broadcast(bc[:], src[:])
            nc.vector.tensor_scalar(out=_r(dst[:]), in0=bc[:], scalar1=t_iota[:],
                                    scalar2=None, op0=OP.is_equal)

        # gather amap columns for each pair: htT = amapT[:, pair_idx]
        htT0x = sbt.tile([128, NH], f32, tag="t")
        htT1x = sbt.tile([128, NH], f32, tag="t")
        nc.gpsimd.ap_gather(htT0x[:].rearrange("c (n o) -> c n o", o=1),
                            amap0[:].rearrange("c (n o) -> c n o", o=1), t_pidx[:],
                            channels=128, num_elems=1024, d=1, num_idxs=NH)
        nc.gpsimd.ap_gather(htT1x[:].rearrange("c (n o) -> c n o", o=1),
                            amap1[:].rearrange("c (n o) -> c n o", o=1), t_pidx[:],
                            channels=128, num_elems=1024, d=1, num_idxs=NH)
        nc.vector.tensor_copy(_r(htT0[:]), htT0x[:])
        nc.vector.tensor_copy(_r(htT1[:]), htT1x[:])

        pu_cm.__exit__(None, None, None)

        # ---------------- pair features + decoder, interleaved per chunk ----
        # for each of the 6 D-chunks: head tanh-arg, tail tanh-arg, then the
        # two decoder groups of that chunk - keeps PE/ACT/DVE pipelined
        ph_cm = tc.tile_pool(name="ph", bufs=4, space="PSUM")
        ph = ph_cm.__enter__()
        pd_cm = tc.tile_pool(name="pd", bufs=2, space="PSUM")
        pd = pd_cm.__enter__()
        po_cm = tc.tile_pool(name="po", bufs=1, space="PSUM")
        po = po_cm.__enter__()
        p_out = po.tile([2, NH], f32, tag="po")
        for k in range(KD):
            cols = slice(k * 128, (k + 1) * 128)
            for (ewt, oh, w2, bp, dstT) in ((ew1, ohhi, t_w2h, t_hbp, hsT),
                                            (et1, ohti, t_w2t, t_tbp, tsT)):
                p_hs = ph.tile([128, NH], f32, tag="ph")
                nc.tensor.matmul(p_hs[:], _r(ewt[:, cols]), _r(oh[:]), start=True, stop=False)
                nc.tensor.matmul(p_hs[:], _r(w2[:, 0, cols]), _r(htT0[:]), start=False, stop=False)
                nc.tensor.matmul(p_hs[:], _r(w2[:, 1, cols]), _r(htT1[:]), start=False, stop=True)
                nc.scalar.activation(_r(dstT[:, k, :]), p_hs[:],
                                     AF.Tanh, bias=bp[:, k:k + 1])
            for half in range(2):
                g = 2 * k + half
                rows = slice(half * 64, (half + 1) * 64)
                p_u = pd.tile([128, NH], f32, tag="pd")
                nc.tensor.matmul(p_u[:], _r(t_wdec[rows, g, :]), _r(tsT[rows, k, :]))
                v = sbt.tile([128, NH], bf16, tag="v")
                nc.vector.tensor_mul(v[0:64, :], p_u[0:64, :], hsT[rows, k, :])
                nc.vector.tensor_mul(v[64:128, :], p_u[64:128, :], hsT[rows, k, :])
                nc.tensor.matmul(p_out[:], t_smat[:], v[:],
                                 start=(g == 0), stop=(g == G - 1))
        out_sb = sbt.tile([2, NH], f32, tag="out")
        nc.scalar.activation(out_sb[:], p_out[:], AF.Identity, bias=t_decb[:])
        nc.sync.dma_start(y[:], out_sb[:])
        po_cm.__exit__(None, None, None)
        pd_cm.__exit__(None, None, None)
        ph_cm.__exit__(None, None, None)

    nc.compile()
    return nc


def f32r_round(a):
    """Round-to-nearest-even to fp32r (11 mantissa bits), matching the PE."""
    u = np.ascontiguousarray(a, np.float32).view(np.uint32).copy()
    u = (u + (np.uint32(0x7FF) + ((u >> np.uint32(12)) & np.uint32(1)))) & np.uint32(0xFFFFF000)
    return u.view(np.float32)


def _wrap16(idx, n_slots):
    """int16 index layout for gpsimd gathers: wrapped in 16 partitions,
    replicated across the 8 gpsimd cores."""
    out = np.zeros((128, n_slots), np.int16)
    for j, v in enumerate(idx):
        out[np.arange(8) * 16 + j % 16, j // 16] = v
    return out


def pack_inputs(inputs):
    """Build the 8 per-core input maps from the full problem inputs."""
    x = np.asarray(inputs["x"], np.float32)
    entity_pos = np.asarray(inputs["entity_pos"])
    hts = np.asarray(inputs["hts"])

    shared = {}
    shared["iota32"] = np.arange(NE, dtype=np.float32).reshape(NE, 1)
    shared["ident"] = np.eye(NE, dtype=np.float32)
    smat = np.zeros((128, 2), np.float32)
    smat[:64, 0] = 1.0
    smat[64:, 1] = 1.0
    shared["smat"] = smat  # cast below
    shared["ones_r"] = np.ones((1, 128), np.float32)

    def W(name):
        return np.asarray(inputs[name], np.float32)

    shared["enc1_w9"] = W("enc1_w").reshape(64, 9).T.reshape(1, 576).copy()
    shared["enc1_bp"] = W("enc1_b").reshape(64, 1)
    shared["enc2_w9"] = W("enc2_w").reshape(128, 64, 9).transpose(1, 2, 0).copy()
    shared["enc2_bp"] = W("enc2_b").reshape(128, 1)
    shared["bott_w9"] = W("bott_w").reshape(256, 128, 9).transpose(1, 2, 0).copy()
    shared["bott_bp"] = W("bott_b").reshape(2, 128).T.copy()
    shared["ag2_wgp"] = W("ag2_wg").reshape(128, 256).T.reshape(2, 128, 128).transpose(1, 0, 2).copy()
    shared["ag2_wxp"] = W("ag2_wx").reshape(128, 128).T.copy()
    shared["ag2_psip"] = W("ag2_psi").reshape(1, 128).T.copy()
    shared["dec2_w9"] = W("dec2_w").reshape(128, 384, 9).transpose(1, 2, 0).reshape(3, 128, 9, 128).transpose(1, 0, 2, 3).copy()
    shared["dec2_bp"] = W("dec2_b").reshape(128, 1)
    shared["ag1_wgp"] = W("ag1_wg").reshape(64, 128).T.copy()
    shared["ag1_wxp"] = W("ag1_wx").reshape(64, 64).T.copy()
    shared["ag1_psip"] = W("ag1_psi").reshape(1, 64).T.copy()
    d1w = W("dec1_w").reshape(64, 192, 9).transpose(1, 2, 0)   # [192, 9, 64]
    shared["dec1_w9a"] = d1w[:128].copy()
    shared["dec1_w9b"] = d1w[128:].copy()
    shared["dec1_bp"] = W("dec1_b").reshape(64, 1)
    shared["fin_wp"] = W("fin_w").reshape(256, 64).T.copy()
    shared["fin_bp"] = W("fin_b").reshape(2, 128).T.copy()

    head_w = W("head_w"); tail_w = W("tail_w")
    shared["W1h"] = head_w[:D].reshape(KD, 128, D).transpose(1, 0, 2).copy()
    shared["W2h"] = head_w[D:].reshape(2, 128, D).transpose(1, 0, 2).copy()
    shared["W1t"] = tail_w[:D].reshape(KD, 128, D).transpose(1, 0, 2).copy()
    shared["W2t"] = tail_w[D:].reshape(2, 128, D).transpose(1, 0, 2).copy()
    shared["head_bp"] = W("head_b").reshape(KD, 128).T.copy()
    shared["tail_bp"] = W("tail_b").reshape(KD, 128).T.copy()
    wd = W("decoder_w").reshape(G, 64, 64, 2).transpose(2, 0, 3, 1).reshape(64, G, 128)
    shared["wdec"] = np.concatenate([wd, wd], axis=0).copy()   # rows duplicated
    shared["dec_bp"] = W("decoder_b").reshape(2, 1)

    for k in ("enc1_w9", "enc2_w9", "bott_w9", "ag2_wgp", "ag2_wxp", "ag2_psip",
              "dec2_w9", "ag1_wgp", "ag1_wxp", "ag1_psip", "dec1_w9a", "dec1_w9b",
              "fin_wp", "W1h", "W2h", "W1t", "W2t", "wdec"):
        shared[k] = f32r_round(shared[k])
    import ml_dtypes
    shared["smat"] = shared["smat"].astype(ml_dtypes.bfloat16)

    in_maps = []
    for c in range(NCORES):
        b, h = c // 2, c % 2
        m = dict(shared)
        m["x_b"] = np.ascontiguousarray(x[b])
        start = entity_pos[b, :, 0].astype(np.int64)
        idx = np.minimum(start + 1, L - 1).astype(np.int16)
        m["ent_idx"] = _wrap16(idx, 2)
        m["ent_mask"] = (start + 1 < L).astype(np.float32).reshape(NE, 1)
        hi = hts[b, h * NH:(h + 1) * NH, 0].astype(np.int64)
        ti = hts[b, h * NH:(h + 1) * NH, 1].astype(np.int64)
        m["hi_f"] = hi.astype(np.float32).reshape(1, NH)
        m["ti_f"] = ti.astype(np.float32).reshape(1, NH)
        m["pair_idx"] = _wrap16((hi * NE + ti).astype(np.int16), NH // 16)
        in_maps.append(m)
    return in_maps


_NC_CACHE = None


def get_nc():
    global _NC_CACHE
    if _NC_CACHE is None:
        _NC_CACHE = build_nc()
    return _NC_CACHE


def kernel(**inputs):
    nc = get_nc()
    in_maps = pack_inputs(inputs)
    res = run_bass_kernel_spmd(nc, in_maps, core_ids=list(range(NCORES)))
    out = np.empty((B * P, 2), np.float32)
    for c in range(NCORES):
        b, h = c // 2, c % 2
        yc = res.results[c]["y"]                  # [2, NH]
        out[b * P + h * NH:b * P + (h + 1) * NH, :] = yc.T
    return out



# revision 8
# speedup vs baseline: 1.2248x; 1.2248x over previous
"""Trainium2 Bass kernel for nn_CoreferenceResolver (coref UNet + pair decoder).

Sharding: core c handles batch b=c//2 and pair-half h=c%2 (496 of 992 pairs).
The gather/cosine/UNet stages are replicated on the two cores sharing a batch;
the extractor linears and group-bilinear decoder are sharded over pairs.

Weights are packed host-side into three bf16 blobs + two streamed bf16 W1
tensors so the whole weight set moves in ~17 DMAs. The extractor premultiply
is computed in transposed layout (ewT[dcol, ne]) so per-pair entity features
come from gpsimd gathers instead of one-hot matmuls.
"""
import os
import sys

for _p in ("/opt/trn_rl_repo",):
    if os.path.isdir(_p) and _p not in sys.path:
        sys.path.insert(0, _p)

import numpy as np

import concourse.bass as bass
import concourse.tile as tile
from concourse import bacc, mybir
from concourse.bass_utils import run_bass_kernel_spmd

f32 = mybir.dt.float32
i16 = mybir.dt.int16
AF = mybir.ActivationFunctionType
OP = mybir.AluOpType
f32r = mybir.dt.float32r
bf16 = mybir.dt.bfloat16


def _f(ap):
    return ap.bitcast(mybir.dt.float32)


def _r(ap):
    """View an fp32 AP as float32r for full-rate PE streaming."""
    return ap.bitcast(f32r)

B, L, D, H = 4, 1024, 768, 12
NE, P = 32, 992
BLOCK = 64
G = D // BLOCK          # 12 groups
OUT_CH = 256
NCORES = 8
NH = P // 2             # 496 pairs per core
KD = D // 128           # 6 chunks of the D dim

# ---------------------------------------------------------------------------
# blob column layouts (bf16 blobs hold matmul weights; f32 blob holds ident,
# mask and biases; i16 blob holds gather indices)
# ---------------------------------------------------------------------------


def _mk_layout(entries):
    cols = {}
    c = 0
    for name, n in entries:
        cols[name] = (c, n)
        c += n
    return cols, c

BA_COLS, CA = _mk_layout([
    ("enc1", 3 * 64),       # [3(dx), 3(dy)*64] lhsT chunks per dy
    ("enc2", 9 * 128),      # [64, 9, 128]
    ("bott", 9 * 256),      # [128, 9, 256]
    ("ag2wg", 2 * 128),     # [128, 2, 128]
    ("ag2wx", 128),         # [128, 128]
    ("ag2psi", 1),          # [128, 1]
    ("smat", 2),            # [128, 2]
])

BB_COLS, CB = _mk_layout([
    ("dec2", 3 * 9 * 128),  # [128, 3, 9, 128]
    ("ag1wg", 64),          # [128, 64]
    ("ag1wx", 64),          # [64, 64]
    ("ag1psi", 1),          # [64, 1]
    ("d1ph", 4 * 4 * 64),   # [128, 4(phase), 4(cell), 64]
    ("d1att", 9 * 64),      # [64, 9, 64]
    ("fin", 256),           # [64, 256]
])

BC_COLS, CC = _mk_layout([
    ("w2h", 2 * 768),       # [128, 2, 768]
    ("w2t", 2 * 768),       # [128, 2, 768]
    ("wdec", G * 128),      # [128, G, 128]
])

F32_COLS, CF = _mk_layout([
    ("ident", NE),          # [32, 32]
    ("mask", 1),            # [32, 1]
    ("enc1b", 1),           # [64, 1]
    ("enc2b", 1),           # [128, 1]
    ("bottb", 2),           # [128, 2]
    ("dec2b", 1),           # [128, 1]
    ("dec1b", 1),           # [64, 1]
    ("finb", 2),            # [128, 2]
    ("hbp", KD),            # [128, KD]
    ("tbp", KD),            # [128, KD]
    ("decb", 1),            # [2, 1]
])

I16_COLS, CI = _mk_layout([
    ("eidx", 2),            # wrap16, 32 idxs
    ("pidx", NH // 16),     # amap pair gather
    ("hidx", NH // 16),     # hi gather
    ("tidx", NH // 16),     # ti gather
])


def build_nc():
    nc = bacc.Bacc("TRN2", target_bir_lowering=False, debug=False, num_devices=NCORES)

    def inp(name, shape, dt=f32):
        return nc.dram_tensor(name, shape, dt, kind="ExternalInput")

    x_b   = inp("x_b", [L, D])
    i16b  = inp("i16b", [128, CI], i16)
    f32b  = inp("f32b", [128, CF])
    blobA = inp("blobA", [128, CA], bf16)
    blobB = inp("blobB", [128, CB], bf16)
    blobC = inp("blobC", [128, CC], bf16)
    W1h   = inp("W1h", [128, KD, D], bf16)
    W1t   = inp("W1t", [128, KD, D], bf16)

    y = nc.dram_tensor("y", [2, NH], f32, kind="ExternalOutput")

    from contextlib import ExitStack
    with tile.TileContext(nc) as tc, ExitStack() as _ctx:
        sbw = _ctx.enter_context(tc.tile_pool(name="sbw", bufs=1))   # persistent
        sbt = _ctx.enter_context(tc.tile_pool(name="sbt", bufs=3))   # rotating temps

        # ------------- DMA issue: SP queue in schedule order ---------------
        t_i16 = sbw.tile([128, CI], i16, tag="i16")
        nc.sync.dma_start(t_i16[:], i16b[:])
        t_f32 = sbw.tile([128, CF], f32, tag="f32")
        nc.sync.dma_start(t_f32[:], f32b[:])
        t_bA = sbw.tile([128, CA], bf16, tag="bA")
        nc.sync.dma_start(t_bA[:], blobA[:])
        w1h_s = []
        for k in range(KD):
            t = sbw.tile([128, D], bf16, tag=f"w1h{k}")
            nc.sync.dma_start(t[:], W1h[:, k, :])
            w1h_s.append(t)
        t_bB = sbw.tile([128, CB], bf16, tag="bB")
        nc.sync.dma_start(t_bB[:], blobB[:])
        t_bC = sbw.tile([128, CC], bf16, tag="bC")
        nc.sync.dma_start(t_bC[:], blobC[:])
        w1t_s = []
        for k in range(KD):
            t = sbw.tile([128, D], bf16, tag=f"w1t{k}")
            nc.sync.dma_start(t[:], W1t[:, k, :])
            w1t_s.append(t)

        def bA(name, parts=128):
            c0, n = BA_COLS[name]
            return t_bA[0:parts, c0:c0 + n]

        def bB(name, parts=128):
            c0, n = BB_COLS[name]
            return t_bB[0:parts, c0:c0 + n]

        def bC(name, parts=128):
            c0, n = BC_COLS[name]
            return t_bC[0:parts, c0:c0 + n]

        def bF(name, parts=128):
            c0, n = F32_COLS[name]
            return t_f32[0:parts, c0:c0 + n]

        def bI(name):
            c0, n = I16_COLS[name]
            return t_i16[:, c0:c0 + n]

        # ------------- persistent SBUF intermediates -----------------------
        nrmT  = sbw.tile([128, KD, NE], f32, tag="nrmT")
        entT  = sbw.tile([128, KD, NE], bf16, tag="entT")
        IC3   = sbw.tile([3, 34, 34], f32, tag="IC3")
        c1p   = sbw.tile([64, 32, 32], f32, tag="c1p")     # dense
        p1p   = sbw.tile([64, 18, 18], f32, tag="p1p")     # padded
        c2p   = sbw.tile([128, 16, 16], f32, tag="c2p")    # dense
        p2p   = sbw.tile([128, 10, 10], f32, tag="p2p")    # padded
        c3a   = sbw.tile([128, 8, 8], f32, tag="c3a")      # dense
        c3b   = sbw.tile([128, 8, 8], f32, tag="c3b")
        u2p0  = sbw.tile([128, 18, 18], f32, tag="u2p0")   # padded
        u2p1  = sbw.tile([128, 18, 18], f32, tag="u2p1")
        att2p = sbw.tile([128, 18, 18], f32, tag="att2p")
        d2pad = sbw.tile([128, 18, 18], f32, tag="d2pad")
        att1p = sbw.tile([64, 34, 34], f32, tag="att1p")
        d1s   = sbw.tile([64, 32, 32], f32, tag="d1s")     # dense
        amap0 = sbw.tile([128, 1024], f32, tag="amap0")
        amap1 = sbw.tile([128, 1024], f32, tag="amap1")
        ewhg  = sbw.tile([128, NE, KD], f32, tag="ewhg")   # gather src (head)
        ewtg  = sbw.tile([128, NE, KD], f32, tag="ewtg")   # gather src (tail)
        ehg   = sbw.tile([128, NH, KD], f32, tag="ehg")    # gathered head ents
        etg   = sbw.tile([128, NH, KD], f32, tag="etg")
        htT0  = sbw.tile([128, NH], f32, tag="htT0")
        htT1  = sbw.tile([128, NH], f32, tag="htT1")
        hsT   = sbw.tile([128, KD, NH], bf16, tag="hsT")
        tsT   = sbw.tile([128, KD, NH], bf16, tag="tsT")

        # ------------- Pool queue: memsets + entity gather -----------------
        for t in (IC3, p1p, p2p, u2p0, u2p1, att2p, d2pad, att1p):
            nc.gpsimd.memset(t[:], 0.0)
        ent_raw = sbw.tile([128, 1, D], f32, tag="entraw")
        nc.gpsimd.dma_gather(ent_raw[:], x_b[:], bI("eidx"),
                             num_idxs=NE, num_idxs_reg=NE, elem_size=D)
        ent = ent_raw[0:NE, 0, :]

        # ------------- norms -> diag matrices ------------------------------
        sq = sbt.tile([NE, D], f32, tag="sq")
        ss = sbt.tile([NE, 1], f32, tag="ss")
        nc.scalar.activation(sq[:], ent, AF.Square, accum_out=ss[:])
        nrmv = sbt.tile([NE, 1], f32, tag="nrmv")
        nc.scalar.sqrt(nrmv[:], ss[:])
        nc.vector.tensor_single_scalar(nrmv[:], nrmv[:], 1e-13, op=OP.max)
        rinv = sbt.tile([NE, 1], f32, tag="rinv")
        nc.vector.reciprocal(rinv[:], nrmv[:])
        nc.vector.tensor_tensor(out=rinv[:], in0=rinv[:], in1=bF("mask", NE),
                                op=OP.mult)
        diag_r = sbt.tile([NE, NE], f32, tag="diag_r")
        nc.vector.tensor_scalar(out=diag_r[:], in0=bF("ident", NE),
                                scalar1=rinv[:], scalar2=None, op0=OP.mult)
        diag_m = sbt.tile([NE, NE], f32, tag="diag_m")
        nc.vector.tensor_scalar(out=diag_m[:], in0=bF("ident", NE),
                                scalar1=bF("mask", NE), scalar2=None, op0=OP.mult)

        pu_cm = tc.tile_pool(name="pu", bufs=3, space="PSUM")
        pu = pu_cm.__enter__()
        pw_cm = tc.tile_pool(name="pw", bufs=1, space="PSUM")
        pw = pw_cm.__enter__()

        # ------------- transposes: nrmT (cos) + entT (premult) -------------
        for k in range(KD):
            p_t = pu.tile([128, NE], f32, tag="pu")
            nc.tensor.transpose(_r(p_t[:]), _r(ent[:, k * 128:(k + 1) * 128]),
                                _r(diag_r[:]))
            nc.vector.tensor_copy(_r(nrmT[:, k, :]), p_t[:])
        for k in range(KD):
            p_t = pu.tile([128, NE], f32, tag="pu")
            nc.tensor.transpose(_r(p_t[:]), _r(ent[:, k * 128:(k + 1) * 128]),
                                _r(diag_m[:]))
            nc.vector.tensor_copy(entT[:, k, :], p_t[:])

        # ------------- cosine matrix ---------------------------------------
        p_cos = pu.tile([NE, NE], f32, tag="pu")
        for k in range(KD):
            nc.tensor.matmul(p_cos[:], _r(nrmT[:, k, :]), _r(nrmT[:, k, :]),
                             start=(k == 0), stop=(k == KD - 1))

        s_cos = sbt.tile([NE, NE], f32, tag="scos")
        nc.scalar.activation(s_cos[:], p_cos[:], AF.Copy)

        # IC3[dx, r, c] = cos[r-1, c+dx-2] (zero padded): three parallel DMAs
        # on three queues, each copying a column-shifted cos into one partition
        for dx, q in ((0, nc.sync), (1, nc.scalar), (2, nc.gpsimd)):
            # IC3[dx, r, c] valid where 0 <= c+dx-2 < 32: c in [2-dx, 34-dx)
            c_lo = max(0, 2 - dx)
            c_hi = min(34, 34 - dx)
            ncol = c_hi - c_lo
            s_lo = c_lo + dx - 2
            q.dma_start(IC3[dx:dx + 1, 1:33, c_lo:c_lo + ncol],
                        s_cos[:, s_lo:s_lo + ncol])

        # ------------- premultiply ewT (head), interleaved below -----------
        pewh = pw.tile([128, KD, NE], f32, tag="pwh")
        pewt = pw.tile([128, KD, NE], f32, tag="pwt")

        def premult(ws, pew, kc):
            for dc in range(KD):
                nc.tensor.matmul(pew[:, dc, :], ws[kc][:, dc * 128:(dc + 1) * 128],
                                 entT[:, kc, :], start=(kc == 0), stop=(kc == KD - 1),
                                 skip_group_check=True)

        premult(w1h_s, pewh, 0)

        # ------------- enc1: 3 row-tap matmuls x 2 N-halves ----------------
        p_c1 = pu.tile([64, 1024], f32, tag="pu")
        for hh in range(2):
            rows = slice(hh * 16 + 1, hh * 16 + 17)
            for dy in range(3):
                # rhs rows r = y+dy-1 for out y in [hh*16, hh*16+16) -> IC3 rows y+dy
                rr = slice(hh * 16 + dy, hh * 16 + dy + 16)
                nc.tensor.matmul(p_c1[:, hh * 512:(hh + 1) * 512],
                                 bA("enc1", 3)[:, dy * 64:(dy + 1) * 64],
                                 _r(IC3[:, rr, 1:33]),
                                 start=(dy == 0), stop=(dy == 2))
        nc.scalar.activation(_r(c1p[:]), p_c1[:].rearrange("c (h w) -> c h w", h=32, w=32),
                             AF.Relu, bias=bF("enc1b", 64))

        premult(w1h_s, pewh, 1)
        premult(w1h_s, pewh, 2)

        # ------------- pool1 -> p1p interior [64, 16, 16] ------------------
        tmp = sbt.tile([64, 16, 16], f32, tag="t")
        nc.vector.tensor_max(tmp[:], c1p[:, 0:32:2, 0:32:2], c1p[:, 0:32:2, 1:32:2])
        nc.vector.tensor_max(tmp[:], tmp[:], c1p[:, 1:32:2, 0:32:2])
        nc.vector.tensor_max(_r(p1p[:, 1:17, 1:17]), tmp[:], c1p[:, 1:32:2, 1:32:2])

        # ------------- enc2: 9 shifted matmuls K=64 ------------------------
        p_c2 = pu.tile([128, 256], f32, tag="pu")
        e2w = bA("enc2", 64).rearrange("c (t m) -> c t m", t=9)
        for tap in range(9):
            dy, dx = tap // 3, tap % 3
            nc.tensor.matmul(p_c2[:], e2w[:, tap, :],
                             _r(p1p[:, dy:dy + 16, dx:dx + 16]),
                             start=(tap == 0), stop=(tap == 8))
        nc.scalar.activation(_r(c2p[:]), p_c2[:].rearrange("c (h w) -> c h w", h=16, w=16),
                             AF.Relu, bias=bF("enc2b"))

        premult(w1h_s, pewh, 3)
        premult(w1h_s, pewh, 4)
        premult(w1h_s, pewh, 5)

        # ------------- pool2 -> p2p interior [128, 8, 8] -------------------
        tmp2 = sbt.tile([128, 8, 8], f32, tag="t")
        nc.vector.tensor_max(tmp2[:], c2p[:, 0:16:2, 0:16:2], c2p[:, 0:16:2, 1:16:2])
        nc.vector.tensor_max(tmp2[:], tmp2[:], c2p[:, 1:16:2, 0:16:2])
        nc.vector.tensor_max(_r(p2p[:, 1:9, 1:9]), tmp2[:], c2p[:, 1:16:2, 1:16:2])

        # head ewT: psum -> gather-src layout [128, ne, kd]
        nc.vector.tensor_copy(_r(ewhg[:].rearrange("c n k -> c k n")), pewh[:])
        nc.gpsimd.ap_gather(ehg[:], ewhg[:], bI("hidx"),
                            channels=128, num_elems=NE, d=KD, num_idxs=NH)

        # ------------- bottleneck: 9 taps x 2 M-chunks, K=128 --------------
        bw = bA("bott").rearrange("c (t m) -> c t m", t=9)
        for mc, dst in ((0, c3a), (1, c3b)):
            p_c3 = pu.tile([128, 64], f32, tag="pu")
            for tap in range(9):
                dy, dx = tap // 3, tap % 3
                nc.tensor.matmul(p_c3[:], bw[:, tap, mc * 128:(mc + 1) * 128],
                                 _r(p2p[:, dy:dy + 8, dx:dx + 8]),
                                 start=(tap == 0), stop=(tap == 8))
            nc.scalar.activation(dst[:], p_c3[:].rearrange("c (h w) -> c h w", h=8, w=8),
                                 AF.Relu, bias=bF("bottb")[:, mc:mc + 1])

        # ------------- up2 -> u2p interiors --------------------------------
        for src, dst in ((c3a, u2p0), (c3b, u2p1)):
            for i in range(2):
                for j in range(2):
                    nc.vector.tensor_copy(_r(dst[:, 1 + i:17:2, 1 + j:17:2]), src[:])

        # ------------- attention gate 2 (pre-upsample trick) ---------------
        # q2 = wg@c3 on the 8x8 grid; r2 = relu(up2(q2) + wx@c2)
        wg2 = bA("ag2wg").rearrange("c (t m) -> c t m", t=2)
        p_q2 = pu.tile([128, 8, 8], f32, tag="pu")
        nc.tensor.matmul(p_q2[:], wg2[:, 0, :], c3a[:].bitcast(f32r), start=True, stop=False)
        nc.tensor.matmul(p_q2[:], wg2[:, 1, :], c3b[:].bitcast(f32r), start=False, stop=True)
        p_x2 = pu.tile([128, 16, 16], f32, tag="pu")
        nc.tensor.matmul(p_x2[:], bA("ag2wx"), c2p[:].bitcast(f32r))
        r2 = sbt.tile([128, 16, 16], f32, tag="r2")
        q2b = p_q2[:].unsqueeze(2).unsqueeze(4).broadcast_to([128, 8, 2, 8, 2])
        nc.vector.tensor_tensor(out=r2[:].rearrange("c (h a) (w b) -> c h a w b", a=2, b=2),
                                in0=p_x2[:].rearrange("c (h a) (w b) -> c h a w b", a=2, b=2),
                                in1=q2b, op=OP.add)
        nc.vector.tensor_single_scalar(r2[:], r2[:], 0.0, op=OP.max)
        p_g2 = pu.tile([1, 256], f32, tag="pu")
        nc.tensor.matmul(p_g2[:], bA("ag2psi"), _r(r2[:].rearrange("c h w -> c (h w)")))
        a2 = sbt.tile([1, 256], f32, tag="a2")
        nc.scalar.activation(_r(a2[:]), p_g2[:], AF.Sigmoid)
        a2b = sbt.tile([128, 256], f32, tag="a2b")
        nc.gpsimd.partition_broadcast(a2b[:], a2[:])
        nc.vector.tensor_mul(_r(att2p[:, 1:17, 1:17]),
                             a2b[:].rearrange("c (h w) -> c h w", h=16, w=16), c2p[:])

        # ------------- dec2: 9 taps x 3 K-chunks ---------------------------
        p_d2 = pu.tile([128, 256], f32, tag="pu")
        d2w = bB("dec2").rearrange("c (s t m) -> c s t m", s=3, t=9)
        srcs2 = (u2p0, u2p1, att2p)
        n_mm = 0
        for tap in range(9):
            dy, dx = tap // 3, tap % 3
            for kc in range(3):
                nc.tensor.matmul(p_d2[:], d2w[:, kc, tap, :],
                                 _r(srcs2[kc][:, dy:dy + 16, dx:dx + 16]),
                                 start=(n_mm == 0), stop=(n_mm == 26))
                n_mm += 1
        nc.scalar.activation(_r(d2pad[:, 1:17, 1:17]),
                             p_d2[:].rearrange("c (h w) -> c h w", h=16, w=16),
                             AF.Relu, bias=bF("dec2b"))

        # ------------- attention gate 1 (pre-upsample trick) ---------------
        p_q1 = pu.tile([64, 16, 16], f32, tag="pu")
        nc.tensor.matmul(p_q1[:], bB("ag1wg"), _r(d2pad[:, 1:17, 1:17]))
        p_x1 = pu.tile([64, 1024], f32, tag="pu")
        c1v = c1p[:].rearrange("c h w -> c (h w)")
        for hh in range(2):
            nc.tensor.matmul(p_x1[:, hh * 512:(hh + 1) * 512], bB("ag1wx", 64),
                             _r(c1v[:, hh * 512:(hh + 1) * 512]),
                             start=True, stop=True, skip_group_check=True)
        r1 = sbt.tile([64, 32, 32], f32, tag="r1")
        q1b = p_q1[:].unsqueeze(2).unsqueeze(4).broadcast_to([64, 16, 2, 16, 2])
        nc.vector.tensor_tensor(out=r1[:].rearrange("c (h a) (w b) -> c h a w b", a=2, b=2),
                                in0=p_x1[:].rearrange("c (h a w b) -> c h a w b", h=16, a=2, w=16, b=2),
                                in1=q1b, op=OP.add)
        nc.vector.tensor_single_scalar(r1[:], r1[:], 0.0, op=OP.max)
        p_g1 = pu.tile([1, 1024], f32, tag="pu")
        r1v = r1[:].rearrange("c h w -> c (h w)")
        for hh in range(2):
            nc.tensor.matmul(p_g1[:, hh * 512:(hh + 1) * 512], bB("ag1psi", 64),
                             _r(r1v[:, hh * 512:(hh + 1) * 512]),
                             start=True, stop=True, skip_group_check=True)
        a1 = sbt.tile([1, 1024], f32, tag="a1")
        nc.scalar.activation(_r(a1[:]), p_g1[:], AF.Sigmoid)
        a1b = sbt.tile([64, 1024], f32, tag="a1b")
        nc.gpsimd.partition_broadcast(a1b[:], a1[:])
        nc.vector.tensor_mul(_r(att1p[:, 1:33, 1:33]),
                             a1b[:].rearrange("c (h w) -> c h w", h=32, w=32), c1p[:])

        # premult tail (W1t stream lands mid/late UNet)
        for kc in range(3):
            premult(w1t_s, pewt, kc)

        # ------------- dec1: 4-phase (u-part 2x2 cells + att 9 taps) -------
        d1ph = bB("d1ph").rearrange("c (p l m) -> c p l m", p=4, l=4)
        d1at = bB("d1att", 64).rearrange("c (t m) -> c t m", t=9)
        for a in range(2):
            for b in range(2):
                ph_i = a * 2 + b
                p_d1 = pu.tile([64, 16, 16], f32, tag="pu")
                n_mm = 0
                for cu in range(2):
                    for cv in range(2):
                        bu = cu + a          # d2pad row base: Du+1
                        bv = cv + b
                        nc.tensor.matmul(p_d1[:], d1ph[:, ph_i, cu * 2 + cv, :],
                                         _r(d2pad[:, bu:bu + 16, bv:bv + 16]),
                                         start=(n_mm == 0), stop=False)
                        n_mm += 1
                for tap in range(9):
                    dy, dx = tap // 3, tap % 3
                    nc.tensor.matmul(p_d1[:], d1at[:, tap, :],
                                     _r(att1p[:, a + dy:a + dy + 31:2,
                                              b + dx:b + dx + 31:2]),
                                     start=False, stop=(tap == 8))
                nc.scalar.activation(_r(d1s[:, a:32:2, b:32:2]), p_d1[:],
                                     AF.Relu, bias=bF("dec1b", 64))
            if a == 0:
                premult(w1t_s, pewt, 3)
                premult(w1t_s, pewt, 4)

        premult(w1t_s, pewt, 5)
        # tail ewT -> gather layout + gather
        nc.vector.tensor_copy(_r(ewtg[:].rearrange("c n k -> c k n")), pewt[:])
        nc.gpsimd.ap_gather(etg[:], ewtg[:], bI("tidx"),
                            channels=128, num_elems=NE, d=KD, num_idxs=NH)

        # ------------- fin 1x1 conv -> amapT [256, 1024] -------------------
        d1v = d1s[:].rearrange("c h w -> c (h w)")
        for mc, dst in ((0, amap0), (1, amap1)):
            p_am = pu.tile([128, 1024], f32, tag="pu")
            for hh in range(2):
                nc.tensor.matmul(p_am[:, hh * 512:(hh + 1) * 512],
                                 bB("fin", 64)[:, mc * 128:(mc + 1) * 128],
                                 _r(d1v[:, hh * 512:(hh + 1) * 512]),
                                 start=True, stop=True, skip_group_check=True)
            nc.scalar.activation(dst[:], p_am[:], AF.Identity,
                                 bias=bF("finb")[:, mc:mc + 1])

        # amap pair gathers
        nc.gpsimd.ap_gather(htT0[:].rearrange("c (n o) -> c n o", o=1),
                            amap0[:].rearrange("c (n o) -> c n o", o=1), bI("pidx"),
                            channels=128, num_elems=1024, d=1, num_idxs=NH)
        nc.gpsimd.ap_gather(htT1[:].rearrange("c (n o) -> c n o", o=1),
                            amap1[:].rearrange("c (n o) -> c n o", o=1), bI("pidx"),
                            channels=128, num_elems=1024, d=1, num_idxs=NH)

        pw_cm.__exit__(None, None, None)
        pu_cm.__exit__(None, None, None)

        # ------------- pair features + decoder -----------------------------
        ph_cm = tc.tile_pool(name="ph", bufs=4, space="PSUM")
        ph = ph_cm.__enter__()
        pd_cm = tc.tile_pool(name="pd", bufs=2, space="PSUM")
        pd = pd_cm.__enter__()
        po_cm = tc.tile_pool(name="po", bufs=1, space="PSUM")
        po = po_cm.__enter__()
        p_out = po.tile([2, NH], f32, tag="po")
        w2h = bC("w2h").rearrange("c (t m) -> c t m", t=2)
        w2t = bC("w2t").rearrange("c (t m) -> c t m", t=2)
        wde = bC("wdec").rearrange("c (g m) -> c g m", g=G)
        for k in range(KD):
            cols = slice(k * 128, (k + 1) * 128)
            for (w2, eg, bp, dstT) in ((w2h, ehg, "hbp", hsT),
                                       (w2t, etg, "tbp", tsT)):
                p_hs = ph.tile([128, NH], f32, tag="ph")
                nc.tensor.matmul(p_hs[:], w2[:, 0, cols], _r(htT0[:]), start=True, stop=False)
                nc.tensor.matmul(p_hs[:], w2[:, 1, cols], _r(htT1[:]), start=False, stop=True)
                hsum = sbt.tile([128, NH], f32, tag="hsum")
                nc.vector.tensor_tensor(out=hsum[:], in0=p_hs[:], in1=eg[:, :, k],
                                        op=OP.add)
                nc.scalar.activation(dstT[:, k, :], hsum[:],
                                     AF.Tanh, bias=bF(bp)[:, k:k + 1])
            for half in range(2):
                g = 2 * k + half
                rows = slice(half * 64, (half + 1) * 64)
                p_u = pd.tile([128, NH], f32, tag="pd")
                nc.tensor.matmul(p_u[:], wde[rows, g, :], tsT[rows, k, :])
                v = sbt.tile([128, NH], bf16, tag="v")
                nc.vector.tensor_mul(v[0:64, :], p_u[0:64, :], hsT[rows, k, :])
                nc.vector.tensor_mul(v[64:128, :], p_u[64:128, :], hsT[rows, k, :])
                nc.tensor.matmul(p_out[:], bA("smat"), v[:],
                                 start=(g == 0), stop=(g == G - 1),
                                 skip_group_check=True)
        out_sb = sbt.tile([2, NH], f32, tag="out")
        nc.scalar.activation(out_sb[:], p_out[:], AF.Identity, bias=bF("decb", 2))
        nc.sync.dma_start(y[:], out_sb[:])
        po_cm.__exit__(None, None, None)
        pd_cm.__exit__(None, None, None)
        ph_cm.__exit__(None, None, None)

    nc.compile()
    return nc


def _wrap16(idx, n_slots):
    """int16 index layout for gpsimd gathers: wrapped in 16 partitions,
    replicated across the 8 gpsimd cores."""
    out = np.zeros((128, n_slots), np.int16)
    for j, v in enumerate(idx):
        out[np.arange(8) * 16 + j % 16, j // 16] = v
    return out


def pack_inputs(inputs):
    """Build the 8 per-core input maps from the full problem inputs."""
    import ml_dtypes
    x = np.asarray(inputs["x"], np.float32)
    entity_pos = np.asarray(inputs["entity_pos"])
    hts = np.asarray(inputs["hts"])

    def W(name):
        return np.asarray(inputs[name], np.float32)

    def blob(layout, ncols, parts_map):
        b = np.zeros((128, ncols), np.float32)
        for name, (arr) in parts_map.items():
            c0, n = layout[name]
            p = arr.shape[0]
            b[0:p, c0:c0 + n] = arr.reshape(p, n)
        return b

    shared = {}
    # blobA
    e1 = W("enc1_w").reshape(64, 9)            # [c, dy*3+dx]
    enc1 = np.zeros((3, 3 * 64), np.float32)   # [dx, dy*64+c]
    for dy in range(3):
        for dx in range(3):
            enc1[dx, dy * 64:(dy + 1) * 64] = e1[:, dy * 3 + dx]
    smat = np.zeros((128, 2), np.float32)
    smat[:64, 0] = 1.0
    smat[64:, 1] = 1.0
    blobA = blob(BA_COLS, CA, {
        "enc1": enc1,
        "enc2": W("enc2_w").reshape(128, 64, 9).transpose(1, 2, 0).copy(),
        "bott": W("bott_w").reshape(256, 128, 9).transpose(1, 2, 0).copy(),
        "ag2wg": W("ag2_wg").reshape(128, 256).T.reshape(2, 128, 128).transpose(1, 0, 2).copy(),
        "ag2wx": W("ag2_wx").reshape(128, 128).T.copy(),
        "ag2psi": W("ag2_psi").reshape(1, 128).T.copy(),
        "smat": smat,
    })
    # blobB
    d1w = W("dec1_w")                          # [64, 192, 3, 3]
    du = d1w[:, 0:128]                         # u-part [64, 128, 3, 3]
    d1ph = np.zeros((128, 4, 4, 64), np.float32)
    taps_u = {(0, 0): [0], (0, 1): [1, 2], (1, 0): [0, 1], (1, 1): [2]}
    for a in range(2):
        for b_ in range(2):
            for cu in range(2):
                for cv in range(2):
                    acc = np.zeros((128, 64), np.float32)
                    for dy in taps_u[(a, cu)]:
                        for dx in taps_u[(b_, cv)]:
                            acc += du[:, :, dy, dx].T
                    d1ph[:, a * 2 + b_, cu * 2 + cv, :] = acc
    blobB = blob(BB_COLS, CB, {
        "dec2": W("dec2_w").reshape(128, 384, 9).transpose(1, 2, 0)
                .reshape(3, 128, 9, 128).transpose(1, 0, 2, 3).copy(),
        "ag1wg": W("ag1_wg").reshape(64, 128).T.copy(),
        "ag1wx": W("ag1_wx").reshape(64, 64).T.copy(),
        "ag1psi": W("ag1_psi").reshape(1, 64).T.copy(),
        "d1ph": d1ph,
        "d1att": d1w[:, 128:192].reshape(64, 64, 9).transpose(1, 2, 0).copy(),
        "fin": W("fin_w").reshape(256, 64).T.copy(),
    })
    # blobC
    head_w = W("head_w"); tail_w = W("tail_w")
    wd = W("decoder_w").reshape(G, 64, 64, 2).transpose(2, 0, 3, 1).reshape(64, G, 128)
    blobC = blob(BC_COLS, CC, {
        "w2h": head_w[D:].reshape(2, 128, D).transpose(1, 0, 2).copy(),
        "w2t": tail_w[D:].reshape(2, 128, D).transpose(1, 0, 2).copy(),
        "wdec": np.concatenate([wd, wd], axis=0).copy(),
    })
    for k, v in (("blobA", blobA), ("blobB", blobB), ("blobC", blobC)):
        shared[k] = v.astype(ml_dtypes.bfloat16)
    shared["W1h"] = head_w[:D].reshape(KD, 128, D).transpose(1, 0, 2).astype(ml_dtypes.bfloat16)
    shared["W1t"] = tail_w[:D].reshape(KD, 128, D).transpose(1, 0, 2).astype(ml_dtypes.bfloat16)

    # f32 blob (per-core mask differs; built per core below)
    f32_shared = {
        "ident": np.eye(NE, dtype=np.float32),
        "enc1b": W("enc1_b").reshape(64, 1),
        "enc2b": W("enc2_b").reshape(128, 1),
        "bottb": W("bott_b").reshape(2, 128).T.copy(),
        "dec2b": W("dec2_b").reshape(128, 1),
        "dec1b": W("dec1_b").reshape(64, 1),
        "finb": W("fin_b").reshape(2, 128).T.copy(),
        "hbp": W("head_b").reshape(KD, 128).T.copy(),
        "tbp": W("tail_b").reshape(KD, 128).T.copy(),
        "decb": W("decoder_b").reshape(2, 1),
    }

    in_maps = []
    for c in range(NCORES):
        b, h = c // 2, c % 2
        m = dict(shared)
        m["x_b"] = np.ascontiguousarray(x[b])
        start = entity_pos[b, :, 0].astype(np.int64)
        idx = np.minimum(start + 1, L - 1).astype(np.int16)
        mask = (start + 1 < L).astype(np.float32).reshape(NE, 1)
        fm = dict(f32_shared)
        fm["mask"] = mask
        f32b = np.zeros((128, CF), np.float32)
        for name, arr in fm.items():
            c0, n = F32_COLS[name]
            p = arr.shape[0]
            f32b[0:p, c0:c0 + n] = arr.reshape(p, n)
        m["f32b"] = f32b
        hi = hts[b, h * NH:(h + 1) * NH, 0].astype(np.int64)
        ti = hts[b, h * NH:(h + 1) * NH, 1].astype(np.int64)
        i16bl = np.zeros((128, CI), np.int16)
        for name, arr in (("eidx", _wrap16(idx, 2)),
                          ("pidx", _wrap16((hi * NE + ti).astype(np.int16), NH // 16)),
                          ("hidx", _wrap16(hi.astype(np.int16), NH // 16)),
                          ("tidx", _wrap16(ti.astype(np.int16), NH // 16))):
            c0, n = I16_COLS[name]
            i16bl[:, c0:c0 + n] = arr
        m["i16b"] = i16bl
        in_maps.append(m)
    return in_maps


_NC_CACHE = None


def get_nc():
    global _NC_CACHE
    if _NC_CACHE is None:
        _NC_CACHE = build_nc()
    return _NC_CACHE


def kernel(**inputs):
    nc = get_nc()
    in_maps = pack_inputs(inputs)
    res = run_bass_kernel_spmd(nc, in_maps, core_ids=list(range(NCORES)))
    out = np.empty((B * P, 2), np.float32)
    for c in range(NCORES):
        b, h = c // 2, c % 2
        yc = res.results[c]["y"]                  # [2, NH]
        out[b * P + h * NH:b * P + (h + 1) * NH, :] = yc.T
    return out


# revision 9
# speedup vs baseline: 1.4080x; 1.1496x over previous
"""Trainium2 Bass kernel for nn_CoreferenceResolver (coref UNet + pair decoder).

Sharding: core c handles batch b=c//2 and pair-half h=c%2 (496 of 992 pairs).
The gather/cosine/UNet stages are replicated on the two cores sharing a batch;
the extractor linears and group-bilinear decoder are sharded over pairs.

Weights are packed host-side into bf16 blobs + two streamed bf16 W1 tensors so
the whole weight set moves in ~20 DMAs. The cosine matrix is computed as a
masked gram matrix (entity norms come off its diagonal), enc1 runs on a
3-partition column im2col, dec1 uses a 4-phase decomposition of the upsample
conv, and the attention gates apply their 1x1 convs before upsampling.
"""
import os
import sys

for _p in ("/opt/trn_rl_repo",):
    if os.path.isdir(_p) and _p not in sys.path:
        sys.path.insert(0, _p)

import numpy as np

import concourse.bass as bass
import concourse.tile as tile
from concourse import bacc, mybir
from concourse.bass_utils import run_bass_kernel_spmd

f32 = mybir.dt.float32
i16 = mybir.dt.int16
AF = mybir.ActivationFunctionType
OP = mybir.AluOpType
f32r = mybir.dt.float32r
bf16 = mybir.dt.bfloat16


def _r(ap):
    """View an fp32 AP as float32r for full-rate PE streaming."""
    return ap.bitcast(f32r)

B, L, D, H = 4, 1024, 768, 12
NE, P = 32, 992
BLOCK = 64
G = D // BLOCK          # 12 groups
OUT_CH = 256
NCORES = 8
NH = P // 2             # 496 pairs per core
KD = D // 128           # 6 chunks of the D dim


def _mk_layout(entries):
    cols = {}
    c = 0
    for name, n in entries:
        cols[name] = (c, n)
        c += n
    return cols, c

BA_COLS, CA = _mk_layout([
    ("enc1", 3 * 64),       # [3(dx), 3(dy)*64]
    ("enc2", 9 * 128),      # [64, 9, 128]
    ("bott", 9 * 256),      # [128, 9, 256]
    ("ag2wg", 2 * 128),     # [128, 2, 128]
    ("ag2wx", 128),         # [128, 128]
    ("ag2psi", 1),          # [128, 1]
    ("smat", 2),            # [128, 2]
])

BB_COLS, CB = _mk_layout([
    ("dec2", 3 * 9 * 128),  # [128, 3, 9, 128]
    ("ag1wg", 64),          # [128, 64]
    ("ag1wx", 64),          # [64, 64]
    ("ag1psi", 1),          # [64, 1]
    ("d1ph", 4 * 4 * 64),   # [128, 4(phase), 4(cell), 64]
    ("d1att", 9 * 64),      # [64, 9, 64]
    ("fin", 256),           # [64, 256]
])

BC_COLS, CC = _mk_layout([
    ("w2h", 2 * 768),       # [128, 2, 768]
    ("w2t", 2 * 768),       # [128, 2, 768]
    ("wdec", G * 128),      # [128, G, 128]
])

F32_COLS, CF = _mk_layout([
    ("ident", NE),          # [32, 32]
    ("mask", 1),            # [32, 1]
    ("enc1b", 1),           # [64, 1]
    ("enc2b", 1),           # [128, 1]
    ("bottb", 2),           # [128, 2]
    ("dec2b", 1),           # [128, 1]
    ("dec1b", 1),           # [64, 1]
    ("finb", 2),            # [128, 2]
    ("hbp", KD),            # [128, KD]
    ("tbp", KD),            # [128, KD]
    ("decb", 1),            # [2, 1]
    ("iota", 1),            # [32, 1]
])

FC_COLS, CFC = _mk_layout([
    ("hi_f", NH),           # [1, NH]
    ("ti_f", NH),           # [1, NH]
])

I16_COLS, CI = _mk_layout([
    ("pidx", NH // 16),     # amap pair gather
])


def build_nc():
    nc = bacc.Bacc("TRN2", target_bir_lowering=False, debug=False, num_devices=NCORES)

    def inp(name, shape, dt=f32):
        return nc.dram_tensor(name, shape, dt, kind="ExternalInput")

    ent_in = inp("ent_in", [NE, D])
    i16b  = inp("i16b", [128, CI], i16)
    f32b  = inp("f32b", [128, CF])
    f32c  = inp("f32c", [1, CFC])
    blobA = inp("blobA", [128, CA], bf16)
    blobB = inp("blobB", [128, CB], bf16)
    blobC = inp("blobC", [128, CC], bf16)
    W1h   = inp("W1h", [128, KD, D], bf16)
    W1t   = inp("W1t", [128, KD, D], bf16)

    y = nc.dram_tensor("y", [2, NH], f32, kind="ExternalOutput")

    from contextlib import ExitStack
    with tile.TileContext(nc) as tc, ExitStack() as _ctx:
        sbw = _ctx.enter_context(tc.tile_pool(name="sbw", bufs=1))   # persistent
        sbt = _ctx.enter_context(tc.tile_pool(name="sbt", bufs=3))   # rotating temps

        # ------------- DMA issue: SP queue in schedule order ---------------
        t_ent = sbw.tile([NE, D], f32, tag="ent")
        nc.sync.dma_start(t_ent[:], ent_in[:])
        t_i16 = sbw.tile([128, CI], i16, tag="i16")
        nc.sync.dma_start(t_i16[:], i16b[:])
        t_f32 = sbw.tile([128, CF], f32, tag="f32")
        nc.sync.dma_start(t_f32[:], f32b[:])
        t_bA = sbw.tile([128, CA], bf16, tag="bA")
        nc.sync.dma_start(t_bA[:], blobA[:])
        t_fc = sbw.tile([1, CFC], f32, tag="fc")
        nc.sync.dma_start(t_fc[:], f32c[:])
        w1h_s = []
        for k in range(KD):
            t = sbw.tile([128, D], bf16, tag=f"w1h{k}")
            nc.sync.dma_start(t[:], W1h[:, k, :])
            w1h_s.append(t)
        t_bB = sbw.tile([128, CB], bf16, tag="bB")
        nc.sync.dma_start(t_bB[:], blobB[:])
        t_bC = sbw.tile([128, CC], bf16, tag="bC")
        nc.sync.dma_start(t_bC[:], blobC[:])
        w1t_s = []
        for k in range(KD):
            t = sbw.tile([128, D], bf16, tag=f"w1t{k}")
            nc.sync.dma_start(t[:], W1t[:, k, :])
            w1t_s.append(t)

        def bA(name, parts=128):
            c0, n = BA_COLS[name]
            return t_bA[0:parts, c0:c0 + n]

        def bB(name, parts=128):
            c0, n = BB_COLS[name]
            return t_bB[0:parts, c0:c0 + n]

        def bC(name, parts=128):
            c0, n = BC_COLS[name]
            return t_bC[0:parts, c0:c0 + n]

        def bF(name, parts=128):
            c0, n = F32_COLS[name]
            return t_f32[0:parts, c0:c0 + n]

        # ------------- persistent SBUF intermediates -----------------------
        entT  = sbw.tile([128, KD, NE], bf16, tag="entT")
        gram  = sbw.tile([NE, NE], f32, tag="gram")
        s_cos = sbw.tile([NE, NE], f32, tag="scos")
        IC3   = sbw.tile([3, 34, 34], f32, tag="IC3")
        c1p   = sbw.tile([64, 32, 32], f32, tag="c1p")     # dense
        p1p   = sbw.tile([64, 18, 18], f32, tag="p1p")     # padded
        c2p   = sbw.tile([128, 16, 16], f32, tag="c2p")    # dense
        p2p   = sbw.tile([128, 10, 10], f32, tag="p2p")    # padded
        c3a   = sbw.tile([128, 8, 8], f32, tag="c3a")      # dense
        c3b   = sbw.tile([128, 8, 8], f32, tag="c3b")
        u2p0  = sbw.tile([128, 18, 18], f32, tag="u2p0")   # padded
        u2p1  = sbw.tile([128, 18, 18], f32, tag="u2p1")
        att2p = sbw.tile([128, 18, 18], f32, tag="att2p")
        d2pad = sbw.tile([128, 18, 18], f32, tag="d2pad")
        att1p = sbw.tile([64, 34, 34], f32, tag="att1p")
        d1s   = sbw.tile([64, 32, 32], f32, tag="d1s")     # dense
        amap0 = sbw.tile([128, 1024], f32, tag="amap0")
        amap1 = sbw.tile([128, 1024], f32, tag="amap1")
        ew1   = sbw.tile([NE, D], f32, tag="ew1")
        et1   = sbw.tile([NE, D], f32, tag="et1")
        ohhi  = sbw.tile([NE, NH], f32, tag="ohhi")
        ohti  = sbw.tile([NE, NH], f32, tag="ohti")
        htT0  = sbw.tile([128, NH], f32, tag="htT0")
        htT1  = sbw.tile([128, NH], f32, tag="htT1")
        hsT   = sbw.tile([128, KD, NH], bf16, tag="hsT")
        tsT   = sbw.tile([128, KD, NH], bf16, tag="tsT")

        # ------------- Pool queue: memsets, later broadcasts/gathers -------
        for t in (IC3, p1p, p2p, u2p0, u2p1, att2p, d2pad, att1p):
            nc.gpsimd.memset(t[:], 0.0)

        # ------------- diag(mask) ------------------------------------------
        diag_m = sbt.tile([NE, NE], f32, tag="diag_m")
        nc.vector.tensor_scalar(out=diag_m[:], in0=bF("ident", NE),
                                scalar1=bF("mask", NE), scalar2=None, op0=OP.mult)

        pu_cm = tc.tile_pool(name="pu", bufs=3, space="PSUM")
        pu = pu_cm.__enter__()
        pw_cm = tc.tile_pool(name="pw", bufs=1, space="PSUM")
        pw = pw_cm.__enter__()

        # ------------- transposes (masked raw entities) --------------------
        for k in range(KD):
            p_t = pu.tile([128, NE], f32, tag="pu")
            nc.tensor.transpose(_r(p_t[:]), _r(t_ent[:, k * 128:(k + 1) * 128]),
                                _r(diag_m[:]))
            nc.vector.tensor_copy(entT[:, k, :], p_t[:])

        # ------------- gram + cosine ---------------------------------------
        p_g = pu.tile([NE, NE], f32, tag="pu")
        for k in range(KD):
            nc.tensor.matmul(p_g[:], entT[:, k, :], entT[:, k, :],
                             start=(k == 0), stop=(k == KD - 1))
        nc.vector.tensor_copy(gram[:], p_g[:])
        # norms off the diagonal: ss = sum(gram * I)
        dd = sbt.tile([NE, NE], f32, tag="dd")
        nc.vector.tensor_mul(dd[:], gram[:], bF("ident", NE))
        ss = sbt.tile([NE, 1], f32, tag="ss")
        nc.vector.reduce_sum(ss[:], dd[:], axis=mybir.AxisListType.X)
        nrmv = sbt.tile([NE, 1], f32, tag="nrmv")
        nc.scalar.sqrt(nrmv[:], ss[:])
        nc.vector.tensor_single_scalar(nrmv[:], nrmv[:], 1e-13, op=OP.max)
        rinv = sbt.tile([NE, 1], f32, tag="rinv")
        nc.vector.reciprocal(rinv[:], nrmv[:])
        nc.vector.tensor_tensor(out=rinv[:], in0=rinv[:], in1=bF("mask", NE),
                                op=OP.mult)
        diag_r = sbt.tile([NE, NE], f32, tag="diag_r")
        nc.vector.tensor_scalar(out=diag_r[:], in0=bF("ident", NE),
                                scalar1=rinv[:], scalar2=None, op0=OP.mult)
        # cos = D * (gram^T * D): transpose-with-diag then row scale
        p_ct = pu.tile([NE, NE], f32, tag="pu")
        nc.tensor.transpose(_r(p_ct[:]), _r(gram[:]), _r(diag_r[:]))
        nc.vector.tensor_scalar(out=s_cos[:], in0=p_ct[:],
                                scalar1=rinv[:], scalar2=None, op0=OP.mult)

        # IC3[dx, r, c] = cos[r-1, c+dx-2] (zero padded)
        for dx, q in ((0, nc.scalar), (1, nc.scalar), (2, nc.gpsimd)):
            c_lo = max(0, 2 - dx)
            c_hi = min(34, 34 - dx)
            ncol = c_hi - c_lo
            s_lo = c_lo + dx - 2
            q.dma_start(IC3[dx:dx + 1, 1:33, c_lo:c_lo + ncol],
                        s_cos[:, s_lo:s_lo + ncol])

        # ------------- premultiply ew = ent_masked @ W1 --------------------
        p_ew = pw.tile([NE, D], f32, tag="pw")

        def premult(ws, kc):
            for n0, n1 in ((0, 512), (512, 768)):
                nc.tensor.matmul(p_ew[:, n0:n1], entT[:, kc, :],
                                 ws[kc][:, n0:n1],
                                 start=(kc == 0), stop=(kc == KD - 1),
                                 skip_group_check=True)

        premult(w1h_s, 0)
        premult(w1h_s, 1)

        # ------------- enc1: 3 row-tap matmuls x 2 N-halves ----------------
        p_c1 = pu.tile([64, 1024], f32, tag="pu")
        for hh in range(2):
            for dy in range(3):
                rr = slice(hh * 16 + dy, hh * 16 + dy + 16)
                nc.tensor.matmul(p_c1[:, hh * 512:(hh + 1) * 512],
                                 bA("enc1", 3)[:, dy * 64:(dy + 1) * 64],
                                 _r(IC3[:, rr, 1:33]),
                                 start=(dy == 0), stop=(dy == 2),
                                 skip_group_check=True)
        nc.scalar.activation(_r(c1p[:]),
                             p_c1[:].rearrange("c (h w) -> c h w", h=32, w=32),
                             AF.Relu, bias=bF("enc1b", 64))

        premult(w1h_s, 2)
        premult(w1h_s, 3)

        # ------------- pool1 -> p1p interior [64, 16, 16] ------------------
        tmp = sbt.tile([64, 16, 16], f32, tag="t")
        nc.vector.tensor_max(tmp[:], c1p[:, 0:32:2, 0:32:2], c1p[:, 0:32:2, 1:32:2])
        nc.vector.tensor_max(tmp[:], tmp[:], c1p[:, 1:32:2, 0:32:2])
        nc.vector.tensor_max(_r(p1p[:, 1:17, 1:17]), tmp[:], c1p[:, 1:32:2, 1:32:2])

        # ------------- enc2: 9 shifted matmuls K=64 ------------------------
        p_c2 = pu.tile([128, 256], f32, tag="pu")
        e2w = bA("enc2", 64).rearrange("c (t m) -> c t m", t=9)
        for tap in range(9):
            dy, dx = tap // 3, tap % 3
            nc.tensor.matmul(p_c2[:], e2w[:, tap, :],
                             _r(p1p[:, dy:dy + 16, dx:dx + 16]),
                             start=(tap == 0), stop=(tap == 8))
        nc.scalar.activation(_r(c2p[:]),
                             p_c2[:].rearrange("c (h w) -> c h w", h=16, w=16),
                             AF.Relu, bias=bF("enc2b"))

        premult(w1h_s, 4)
        premult(w1h_s, 5)
        nc.scalar.activation(ew1[:], p_ew[:], AF.Copy)

        # one-hots for the pair gather of ew rows
        for (src_c, dst) in (("hi_f", ohhi), ("ti_f", ohti)):
            c0, n = FC_COLS[src_c]
            bc = sbt.tile([NE, NH], f32, tag="bc")
            nc.gpsimd.partition_broadcast(bc[:], t_fc[0:1, c0:c0 + n])
            nc.vector.tensor_scalar(out=_r(dst[:]), in0=bc[:],
                                    scalar1=bF("iota", NE), scalar2=None,
                                    op0=OP.is_equal)

        # ------------- pool2 -> p2p interior [128, 8, 8] -------------------
        tmp2 = sbt.tile([128, 8, 8], f32, tag="t")
        nc.vector.tensor_max(tmp2[:], c2p[:, 0:16:2, 0:16:2], c2p[:, 0:16:2, 1:16:2])
        nc.vector.tensor_max(tmp2[:], tmp2[:], c2p[:, 1:16:2, 0:16:2])
        nc.vector.tensor_max(_r(p2p[:, 1:9, 1:9]), tmp2[:], c2p[:, 1:16:2, 1:16:2])

        # ------------- bottleneck: 9 taps x 2 M-chunks, K=128 --------------
        bw = bA("bott").rearrange("c (t m) -> c t m", t=9)
        for mc, dst in ((0, c3a), (1, c3b)):
            p_c3 = pu.tile([128, 64], f32, tag="pu")
            for tap in range(9):
                dy, dx = tap // 3, tap % 3
                nc.tensor.matmul(p_c3[:], bw[:, tap, mc * 128:(mc + 1) * 128],
                                 _r(p2p[:, dy:dy + 8, dx:dx + 8]),
                                 start=(tap == 0), stop=(tap == 8))
            nc.scalar.activation(dst[:], p_c3[:].rearrange("c (h w) -> c h w", h=8, w=8),
                                 AF.Relu, bias=bF("bottb")[:, mc:mc + 1])

        # ------------- up2 -> u2p interiors --------------------------------
        for src, dst in ((c3a, u2p0), (c3b, u2p1)):
            for i in range(2):
                for j in range(2):
                    nc.vector.tensor_copy(_r(dst[:, 1 + i:17:2, 1 + j:17:2]), src[:])

        # ------------- attention gate 2 (pre-upsample trick) ---------------
        wg2 = bA("ag2wg").rearrange("c (t m) -> c t m", t=2)
        p_q2 = pu.tile([128, 8, 8], f32, tag="pu")
        nc.tensor.matmul(p_q2[:], wg2[:, 0, :], c3a[:].bitcast(f32r), start=True, stop=False)
        nc.tensor.matmul(p_q2[:], wg2[:, 1, :], c3b[:].bitcast(f32r), start=False, stop=True)
        p_x2 = pu.tile([128, 16, 16], f32, tag="pu")
        nc.tensor.matmul(p_x2[:], bA("ag2wx"), c2p[:].bitcast(f32r))
        r2 = sbt.tile([128, 16, 16], f32, tag="r2")
        q2b = p_q2[:].unsqueeze(2).unsqueeze(4).broadcast_to([128, 8, 2, 8, 2])
        nc.vector.tensor_tensor(out=r2[:].rearrange("c (h a) (w b) -> c h a w b", a=2, b=2),
                                in0=p_x2[:].rearrange("c (h a) (w b) -> c h a w b", a=2, b=2),
                                in1=q2b, op=OP.add)
        nc.vector.tensor_single_scalar(r2[:], r2[:], 0.0, op=OP.max)
        p_g2 = pu.tile([1, 256], f32, tag="pu")
        nc.tensor.matmul(p_g2[:], bA("ag2psi"), _r(r2[:].rearrange("c h w -> c (h w)")))
        a2 = sbt.tile([1, 256], f32, tag="a2")
        nc.scalar.activation(_r(a2[:]), p_g2[:], AF.Sigmoid)
        a2b = sbt.tile([128, 256], f32, tag="a2b")
        nc.gpsimd.partition_broadcast(a2b[:], a2[:])
        nc.vector.tensor_mul(_r(att2p[:, 1:17, 1:17]),
                             a2b[:].rearrange("c (h w) -> c h w", h=16, w=16), c2p[:])

        # ------------- dec2: 9 taps x 3 K-chunks ---------------------------
        p_d2 = pu.tile([128, 256], f32, tag="pu")
        d2w = bB("dec2").rearrange("c (s t m) -> c s t m", s=3, t=9)
        srcs2 = (u2p0, u2p1, att2p)
        n_mm = 0
        for tap in range(9):
            dy, dx = tap // 3, tap % 3
            for kc in range(3):
                nc.tensor.matmul(p_d2[:], d2w[:, kc, tap, :],
                                 _r(srcs2[kc][:, dy:dy + 16, dx:dx + 16]),
                                 start=(n_mm == 0), stop=(n_mm == 26))
                n_mm += 1
        nc.scalar.activation(_r(d2pad[:, 1:17, 1:17]),
                             p_d2[:].rearrange("c (h w) -> c h w", h=16, w=16),
                             AF.Relu, bias=bF("dec2b"))

        # ------------- attention gate 1 (pre-upsample trick) ---------------
        p_q1 = pu.tile([64, 16, 16], f32, tag="pu")
        nc.tensor.matmul(p_q1[:], bB("ag1wg"), _r(d2pad[:, 1:17, 1:17]))
        p_x1 = pu.tile([64, 1024], f32, tag="pu")
        c1v = c1p[:].rearrange("c h w -> c (h w)")
        for hh in range(2):
            nc.tensor.matmul(p_x1[:, hh * 512:(hh + 1) * 512], bB("ag1wx", 64),
                             _r(c1v[:, hh * 512:(hh + 1) * 512]),
                             start=True, stop=True, skip_group_check=True)
        r1 = sbt.tile([64, 32, 32], f32, tag="r1")
        q1b = p_q1[:].unsqueeze(2).unsqueeze(4).broadcast_to([64, 16, 2, 16, 2])
        nc.vector.tensor_tensor(out=r1[:].rearrange("c (h a) (w b) -> c h a w b", a=2, b=2),
                                in0=p_x1[:].rearrange("c (h a w b) -> c h a w b", h=16, a=2, w=16, b=2),
                                in1=q1b, op=OP.add)
        nc.vector.tensor_single_scalar(r1[:], r1[:], 0.0, op=OP.max)
        p_g1 = pu.tile([1, 1024], f32, tag="pu")
        r1v = r1[:].rearrange("c h w -> c (h w)")
        for hh in range(2):
            nc.tensor.matmul(p_g1[:, hh * 512:(hh + 1) * 512], bB("ag1psi", 64),
                             _r(r1v[:, hh * 512:(hh + 1) * 512]),
                             start=True, stop=True, skip_group_check=True)
        a1 = sbt.tile([1, 1024], f32, tag="a1")
        nc.scalar.activation(_r(a1[:]), p_g1[:], AF.Sigmoid)
        a1b = sbt.tile([64, 1024], f32, tag="a1b")
        nc.gpsimd.partition_broadcast(a1b[:], a1[:])
        nc.vector.tensor_mul(_r(att1p[:, 1:33, 1:33]),
                             a1b[:].rearrange("c (h w) -> c h w", h=32, w=32), c1p[:])

        # premult tail (W1t stream lands late UNet)
        premult(w1t_s, 0)
        premult(w1t_s, 1)

        # ------------- dec1: 4-phase (u-part 2x2 cells + att 9 taps) -------
        d1ph = bB("d1ph").rearrange("c (p l m) -> c p l m", p=4, l=4)
        d1at = bB("d1att", 64).rearrange("c (t m) -> c t m", t=9)
        for a in range(2):
            for b in range(2):
                ph_i = a * 2 + b
                p_d1 = pu.tile([64, 16, 16], f32, tag="pu")
                n_mm = 0
                for cu in range(2):
                    for cv in range(2):
                        nc.tensor.matmul(p_d1[:], d1ph[:, ph_i, cu * 2 + cv, :],
                                         _r(d2pad[:, cu + a:cu + a + 16,
                                                  cv + b:cv + b + 16]),
                                         start=(n_mm == 0), stop=False)
                        n_mm += 1
                for tap in range(9):
                    dy, dx = tap // 3, tap % 3
                    nc.tensor.matmul(p_d1[:], d1at[:, tap, :],
                                     _r(att1p[:, a + dy:a + dy + 31:2,
                                              b + dx:b + dx + 31:2]),
                                     start=False, stop=(tap == 8))
                nc.scalar.activation(_r(d1s[:, a:32:2, b:32:2]), p_d1[:],
                                     AF.Relu, bias=bF("dec1b", 64))
            if a == 0:
                premult(w1t_s, 2)
                premult(w1t_s, 3)

        premult(w1t_s, 4)
        premult(w1t_s, 5)
        nc.scalar.activation(et1[:], p_ew[:], AF.Copy)

        # ------------- fin 1x1 conv -> amapT [256, 1024] -------------------
        d1v = d1s[:].rearrange("c h w -> c (h w)")
        for mc, dst in ((0, amap0), (1, amap1)):
            p_am = pu.tile([128, 1024], f32, tag="pu")
            for hh in range(2):
                nc.tensor.matmul(p_am[:, hh * 512:(hh + 1) * 512],
                                 bB("fin", 64)[:, mc * 128:(mc + 1) * 128],
                                 _r(d1v[:, hh * 512:(hh + 1) * 512]),
                                 start=True, stop=True, skip_group_check=True)
            nc.scalar.activation(dst[:], p_am[:], AF.Identity,
                                 bias=bF("finb")[:, mc:mc + 1])

        # amap pair gathers
        c0, n = I16_COLS["pidx"]
        pidx = t_i16[:, c0:c0 + n]
        nc.gpsimd.ap_gather(htT0[:].rearrange("c (n o) -> c n o", o=1),
                            amap0[:].rearrange("c (n o) -> c n o", o=1), pidx,
                            channels=128, num_elems=1024, d=1, num_idxs=NH)
        nc.gpsimd.ap_gather(htT1[:].rearrange("c (n o) -> c n o", o=1),
                            amap1[:].rearrange("c (n o) -> c n o", o=1), pidx,
                            channels=128, num_elems=1024, d=1, num_idxs=NH)

        pw_cm.__exit__(None, None, None)
        pu_cm.__exit__(None, None, None)

        # ------------- pair features + decoder -----------------------------
        ph_cm = tc.tile_pool(name="ph", bufs=4, space="PSUM")
        ph = ph_cm.__enter__()
        pd_cm = tc.tile_pool(name="pd", bufs=2, space="PSUM")
        pd = pd_cm.__enter__()
        po_cm = tc.tile_pool(name="po", bufs=1, space="PSUM")
        po = po_cm.__enter__()
        p_out = po.tile([2, NH], f32, tag="po")
        w2h = bC("w2h").rearrange("c (t m) -> c t m", t=2)
        w2t = bC("w2t").rearrange("c (t m) -> c t m", t=2)
        wde = bC("wdec").rearrange("c (g m) -> c g m", g=G)
        for k in range(KD):
            cols = slice(k * 128, (k + 1) * 128)
            for (w2, ewt, oh, bp, dstT) in ((w2h, ew1, ohhi, "hbp", hsT),
                                            (w2t, et1, ohti, "tbp", tsT)):
                p_hs = ph.tile([128, NH], f32, tag="ph")
                nc.tensor.matmul(p_hs[:], _r(ewt[:, cols]), _r(oh[:]),
                                 start=True, stop=False)
                nc.tensor.matmul(p_hs[:], w2[:, 0, cols], _r(htT0[:]),
                                 start=False, stop=False)
                nc.tensor.matmul(p_hs[:], w2[:, 1, cols], _r(htT1[:]),
                                 start=False, stop=True)
                nc.scalar.activation(dstT[:, k, :], p_hs[:],
                                     AF.Tanh, bias=bF(bp)[:, k:k + 1])
            for half in range(2):
                g = 2 * k + half
                rows = slice(half * 64, (half + 1) * 64)
                p_u = pd.tile([128, NH], f32, tag="pd")
                nc.tensor.matmul(p_u[:], wde[rows, g, :], tsT[rows, k, :])
                v = sbt.tile([128, NH], bf16, tag="v")
                nc.vector.tensor_mul(v[0:64, :], p_u[0:64, :], hsT[rows, k, :])
                nc.vector.tensor_mul(v[64:128, :], p_u[64:128, :], hsT[rows, k, :])
                nc.tensor.matmul(p_out[:], bA("smat"), v[:],
                                 start=(g == 0), stop=(g == G - 1),
                                 skip_group_check=True)
        out_sb = sbt.tile([2, NH], f32, tag="out")
        nc.scalar.activation(out_sb[:], p_out[:], AF.Identity, bias=bF("decb", 2))
        nc.sync.dma_start(y[:], out_sb[:])
        po_cm.__exit__(None, None, None)
        pd_cm.__exit__(None, None, None)
        ph_cm.__exit__(None, None, None)

    nc.compile()
    return nc


def _wrap16(idx, n_slots):
    """int16 index layout for gpsimd gathers: wrapped in 16 partitions,
    replicated across the 8 gpsimd cores."""
    out = np.zeros((128, n_slots), np.int16)
    for j, v in enumerate(idx):
        out[np.arange(8) * 16 + j % 16, j // 16] = v
    return out


def pack_inputs(inputs):
    """Build the 8 per-core input maps from the full problem inputs."""
    import ml_dtypes
    x = np.asarray(inputs["x"], np.float32)
    entity_pos = np.asarray(inputs["entity_pos"])
    hts = np.asarray(inputs["hts"])

    def W(name):
        return np.asarray(inputs[name], np.float32)

    def blob(layout, ncols, parts_map):
        b = np.zeros((128, ncols), np.float32)
        for name, arr in parts_map.items():
            c0, n = layout[name]
            p = arr.shape[0]
            b[0:p, c0:c0 + n] = arr.reshape(p, n)
        return b

    shared = {}
    e1 = W("enc1_w").reshape(64, 9)            # [c, dy*3+dx]
    enc1 = np.zeros((3, 3 * 64), np.float32)   # [dx, dy*64+c]
    for dy in range(3):
        for dx in range(3):
            enc1[dx, dy * 64:(dy + 1) * 64] = e1[:, dy * 3 + dx]
    smat = np.zeros((128, 2), np.float32)
    smat[:64, 0] = 1.0
    smat[64:, 1] = 1.0
    blobA = blob(BA_COLS, CA, {
        "enc1": enc1,
        "enc2": W("enc2_w").reshape(128, 64, 9).transpose(1, 2, 0).copy(),
        "bott": W("bott_w").reshape(256, 128, 9).transpose(1, 2, 0).copy(),
        "ag2wg": W("ag2_wg").reshape(128, 256).T.reshape(2, 128, 128).transpose(1, 0, 2).copy(),
        "ag2wx": W("ag2_wx").reshape(128, 128).T.copy(),
        "ag2psi": W("ag2_psi").reshape(1, 128).T.copy(),
        "smat": smat,
    })
    d1w = W("dec1_w")                          # [64, 192, 3, 3]
    du = d1w[:, 0:128]                         # u-part [64, 128, 3, 3]
    d1ph = np.zeros((128, 4, 4, 64), np.float32)
    taps_u = {(0, 0): [0], (0, 1): [1, 2], (1, 0): [0, 1], (1, 1): [2]}
    for a in range(2):
        for b_ in range(2):
            for cu in range(2):
                for cv in range(2):
                    acc = np.zeros((128, 64), np.float32)
                    for dy in taps_u[(a, cu)]:
                        for dx in taps_u[(b_, cv)]:
                            acc += du[:, :, dy, dx].T
                    d1ph[:, a * 2 + b_, cu * 2 + cv, :] = acc
    blobB = blob(BB_COLS, CB, {
        "dec2": W("dec2_w").reshape(128, 384, 9).transpose(1, 2, 0)
                .reshape(3, 128, 9, 128).transpose(1, 0, 2, 3).copy(),
        "ag1wg": W("ag1_wg").reshape(64, 128).T.copy(),
        "ag1wx": W("ag1_wx").reshape(64, 64).T.copy(),
        "ag1psi": W("ag1_psi").reshape(1, 64).T.copy(),
        "d1ph": d1ph,
        "d1att": d1w[:, 128:192].reshape(64, 64, 9).transpose(1, 2, 0).copy(),
        "fin": W("fin_w").reshape(256, 64).T.copy(),
    })
    head_w = W("head_w"); tail_w = W("tail_w")
    wd = W("decoder_w").reshape(G, 64, 64, 2).transpose(2, 0, 3, 1).reshape(64, G, 128)
    blobC = blob(BC_COLS, CC, {
        "w2h": head_w[D:].reshape(2, 128, D).transpose(1, 0, 2).copy(),
        "w2t": tail_w[D:].reshape(2, 128, D).transpose(1, 0, 2).copy(),
        "wdec": np.concatenate([wd, wd], axis=0).copy(),
    })
    for k, v in (("blobA", blobA), ("blobB", blobB), ("blobC", blobC)):
        shared[k] = v.astype(ml_dtypes.bfloat16)
    shared["W1h"] = head_w[:D].reshape(KD, 128, D).transpose(1, 0, 2).astype(ml_dtypes.bfloat16)
    shared["W1t"] = tail_w[:D].reshape(KD, 128, D).transpose(1, 0, 2).astype(ml_dtypes.bfloat16)

    f32_shared = {
        "ident": np.eye(NE, dtype=np.float32),
        "enc1b": W("enc1_b").reshape(64, 1),
        "enc2b": W("enc2_b").reshape(128, 1),
        "bottb": W("bott_b").reshape(2, 128).T.copy(),
        "dec2b": W("dec2_b").reshape(128, 1),
        "dec1b": W("dec1_b").reshape(64, 1),
        "finb": W("fin_b").reshape(2, 128).T.copy(),
        "hbp": W("head_b").reshape(KD, 128).T.copy(),
        "tbp": W("tail_b").reshape(KD, 128).T.copy(),
        "decb": W("decoder_b").reshape(2, 1),
        "iota": np.arange(NE, dtype=np.float32).reshape(NE, 1),
    }

    in_maps = []
    for c in range(NCORES):
        b, h = c // 2, c % 2
        m = dict(shared)
        start = entity_pos[b, :, 0].astype(np.int64)
        idx = np.minimum(start + 1, L - 1)
        mask = (start + 1 < L).astype(np.float32).reshape(NE, 1)
        m["ent_in"] = np.ascontiguousarray(x[b][idx])
        fm = dict(f32_shared)
        fm["mask"] = mask
        f32bl = np.zeros((128, CF), np.float32)
        for name, arr in fm.items():
            c0, n = F32_COLS[name]
            p = arr.shape[0]
            f32bl[0:p, c0:c0 + n] = arr.reshape(p, n)
        m["f32b"] = f32bl
        hi = hts[b, h * NH:(h + 1) * NH, 0].astype(np.int64)
        ti = hts[b, h * NH:(h + 1) * NH, 1].astype(np.int64)
        fcb = np.zeros((1, CFC), np.float32)
        fcb[0, FC_COLS["hi_f"][0]:FC_COLS["hi_f"][0] + NH] = hi
        fcb[0, FC_COLS["ti_f"][0]:FC_COLS["ti_f"][0] + NH] = ti
        m["f32c"] = fcb
        i16bl = np.zeros((128, CI), np.int16)
        c0, n = I16_COLS["pidx"]
        i16bl[:, c0:c0 + n] = _wrap16((hi * NE + ti).astype(np.int16), NH // 16)
        m["i16b"] = i16bl
        in_maps.append(m)
    return in_maps


_NC_CACHE = None


def get_nc():
    global _NC_CACHE
    if _NC_CACHE is None:
        _NC_CACHE = build_nc()
    return _NC_CACHE


def kernel(**inputs):
    nc = get_nc()
    in_maps = pack_inputs(inputs)
    res = run_bass_kernel_spmd(nc, in_maps, core_ids=list(range(NCORES)))
    out = np.empty((B * P, 2), np.float32)
    for c in range(NCORES):
        b, h = c // 2, c % 2
        yc = res.results[c]["y"]                  # [2, NH]
        out[b * P + h * NH:b * P + (h + 1) * NH, :] = yc.T
    return out


# revision 11
# speedup vs baseline: 1.4653x; 1.0407x over previous
"""Trainium2 Bass kernel for nn_CoreferenceResolver (coref UNet + pair decoder).

Sharding: core c handles batch b=c//2 and pair-half h=c%2 (496 of 992 pairs).
The gather/cosine/UNet stages are replicated on the two cores sharing a batch;
the extractor linears and group-bilinear decoder are sharded over pairs.

Weights are packed host-side into bf16 blobs + two streamed bf16 W1 tensors so
the whole weight set moves in ~20 DMAs. The cosine matrix is computed as a
masked gram matrix (entity norms come off its diagonal), enc1 runs on a
3-partition column im2col, dec1 uses a 4-phase decomposition of the upsample
conv, and the attention gates apply their 1x1 convs before upsampling.
"""
import os
import sys

for _p in ("/opt/trn_rl_repo",):
    if os.path.isdir(_p) and _p not in sys.path:
        sys.path.insert(0, _p)

import numpy as np

import concourse.bass as bass
import concourse.tile as tile
from concourse import bacc, mybir
from concourse.bass_utils import run_bass_kernel_spmd

f32 = mybir.dt.float32
i16 = mybir.dt.int16
AF = mybir.ActivationFunctionType
OP = mybir.AluOpType
f32r = mybir.dt.float32r
bf16 = mybir.dt.bfloat16


def _r(ap):
    """View an fp32 AP as float32r for full-rate PE streaming."""
    return ap.bitcast(f32r)

B, L, D, H = 4, 1024, 768, 12
NE, P = 32, 992
BLOCK = 64
G = D // BLOCK          # 12 groups
OUT_CH = 256
NCORES = 8
NH = P // 2             # 496 pairs per core
KD = D // 128           # 6 chunks of the D dim


def _mk_layout(entries):
    cols = {}
    c = 0
    for name, n in entries:
        cols[name] = (c, n)
        c += n
    return cols, c

BA_COLS, CA = _mk_layout([
    ("enc1", 3 * 64),       # [3(dx), 3(dy)*64]
    ("enc2", 9 * 128),      # [64, 9, 128]
    ("bott", 9 * 256),      # [128, 9, 256]
    ("ag2wg", 2 * 128),     # [128, 2, 128]
    ("ag2wx", 128),         # [128, 128]
    ("ag2psi", 1),          # [128, 1]
    ("smat", 2),            # [128, 2]
])

BB_COLS, CB = _mk_layout([
    ("dec2", 3 * 9 * 128),  # [128, 3, 9, 128]
    ("ag1wg", 64),          # [128, 64]
    ("ag1wx", 64),          # [64, 64]
    ("ag1psi", 1),          # [64, 1]
    ("d1ph", 4 * 4 * 64),   # [128, 4(phase), 4(cell), 64]
    ("d1att", 9 * 64),      # [64, 9, 64]
    ("fin", 256),           # [64, 256]
])

BC_COLS, CC = _mk_layout([
    ("w2h", 2 * 768),       # [128, 2, 768]
    ("w2t", 2 * 768),       # [128, 2, 768]
    ("wdec", G * 128),      # [128, G, 128]
])

F32_COLS, CF = _mk_layout([
    ("ident", NE),          # [32, 32]
    ("mask", 1),            # [32, 1]
    ("enc1b", 1),           # [64, 1]
    ("enc2b", 1),           # [128, 1]
    ("bottb", 2),           # [128, 2]
    ("dec2b", 1),           # [128, 1]
    ("dec1b", 1),           # [64, 1]
    ("finb", 2),            # [128, 2]
    ("hbp", KD),            # [128, KD]
    ("tbp", KD),            # [128, KD]
    ("decb", 1),            # [2, 1]
    ("iota", 1),            # [32, 1]
    ("ones", 128),          # [1, 128]
])

FC_COLS, CFC = _mk_layout([
    ("hi_f", NH),           # [1, NH]
    ("ti_f", NH),           # [1, NH]
])

I16_COLS, CI = _mk_layout([
    ("pidx", NH // 16),     # amap pair gather
])


def build_nc():
    nc = bacc.Bacc("TRN2", target_bir_lowering=False, debug=False, num_devices=NCORES)

    def inp(name, shape, dt=f32):
        return nc.dram_tensor(name, shape, dt, kind="ExternalInput")

    ent_in = inp("ent_in", [NE, D])
    i16b  = inp("i16b", [128, CI], i16)
    f32b  = inp("f32b", [128, CF])
    f32c  = inp("f32c", [1, CFC])
    blobA = inp("blobA", [128, CA], bf16)
    blobB = inp("blobB", [128, CB], bf16)
    blobC = inp("blobC", [128, CC], bf16)
    W1h   = inp("W1h", [128, KD, D], bf16)
    W1t   = inp("W1t", [128, KD, D], bf16)

    y = nc.dram_tensor("y", [2, NH], f32, kind="ExternalOutput")

    from contextlib import ExitStack
    with tile.TileContext(nc) as tc, ExitStack() as _ctx:
        sbw = _ctx.enter_context(tc.tile_pool(name="sbw", bufs=1))   # persistent
        sbt = _ctx.enter_context(tc.tile_pool(name="sbt", bufs=3))   # rotating temps

        # ------------- DMA issue: SP queue in schedule order ---------------
        t_ent = sbw.tile([NE, D], f32, tag="ent")
        nc.sync.dma_start(t_ent[:], ent_in[:])
        t_i16 = sbw.tile([128, CI], i16, tag="i16")
        nc.sync.dma_start(t_i16[:], i16b[:])
        t_f32 = sbw.tile([128, CF], f32, tag="f32")
        nc.sync.dma_start(t_f32[:], f32b[:])
        t_bA = sbw.tile([128, CA], bf16, tag="bA")
        nc.sync.dma_start(t_bA[:], blobA[:])
        t_fc = sbw.tile([1, CFC], f32, tag="fc")
        nc.sync.dma_start(t_fc[:], f32c[:])
        w1h_s = []
        for k in range(KD):
            t = sbw.tile([128, D], bf16, tag=f"w1h{k}")
            nc.sync.dma_start(t[:], W1h[:, k, :])
            w1h_s.append(t)
        t_bB = sbw.tile([128, CB], bf16, tag="bB")
        nc.sync.dma_start(t_bB[:], blobB[:])
        t_bC = sbw.tile([128, CC], bf16, tag="bC")
        nc.sync.dma_start(t_bC[:], blobC[:])
        w1t_s = []
        for k in range(KD):
            t = sbw.tile([128, D], bf16, tag=f"w1t{k}")
            nc.sync.dma_start(t[:], W1t[:, k, :])
            w1t_s.append(t)

        def bA(name, parts=128):
            c0, n = BA_COLS[name]
            return t_bA[0:parts, c0:c0 + n]

        def bB(name, parts=128):
            c0, n = BB_COLS[name]
            return t_bB[0:parts, c0:c0 + n]

        def bC(name, parts=128):
            c0, n = BC_COLS[name]
            return t_bC[0:parts, c0:c0 + n]

        def bF(name, parts=128):
            c0, n = F32_COLS[name]
            return t_f32[0:parts, c0:c0 + n]

        # ------------- persistent SBUF intermediates -----------------------
        entT  = sbw.tile([128, KD, NE], bf16, tag="entT")
        gram  = sbw.tile([NE, NE], f32, tag="gram")
        s_cos = sbw.tile([NE, NE], f32, tag="scos")
        IC3   = sbw.tile([3, 34, 34], f32, tag="IC3")
        c1p   = sbw.tile([64, 32, 32], f32, tag="c1p")     # dense
        p1p   = sbw.tile([64, 18, 18], f32, tag="p1p")     # padded
        c2p   = sbw.tile([128, 16, 16], f32, tag="c2p")    # dense
        p2p   = sbw.tile([128, 10, 10], bf16, tag="p2p")   # padded
        c3a   = sbw.tile([128, 8, 8], bf16, tag="c3a")     # dense
        c3b   = sbw.tile([128, 8, 8], bf16, tag="c3b")
        u2p0  = sbw.tile([128, 18, 18], f32, tag="u2p0")   # padded
        u2p1  = sbw.tile([128, 18, 18], f32, tag="u2p1")
        att2p = sbw.tile([128, 18, 18], f32, tag="att2p")
        d2pad = sbw.tile([128, 18, 18], f32, tag="d2pad")
        att1p = sbw.tile([64, 34, 34], f32, tag="att1p")
        d1s   = sbw.tile([64, 32, 32], f32, tag="d1s")     # dense
        amap0 = sbw.tile([128, 1024], f32, tag="amap0")
        amap1 = sbw.tile([128, 1024], f32, tag="amap1")
        ew1   = sbw.tile([NE, D], f32, tag="ew1")
        et1   = sbw.tile([NE, D], f32, tag="et1")
        ohhi  = sbw.tile([NE, NH], f32, tag="ohhi")
        ohti  = sbw.tile([NE, NH], f32, tag="ohti")
        htT0  = sbw.tile([128, NH], f32, tag="htT0")
        htT1  = sbw.tile([128, NH], f32, tag="htT1")
        hsT   = sbw.tile([128, KD, NH], bf16, tag="hsT")
        tsT   = sbw.tile([128, KD, NH], bf16, tag="tsT")

        # ------------- Pool queue: memsets, later broadcasts/gathers -------
        for t in (IC3, p1p, p2p, u2p0, u2p1, att2p, d2pad, att1p):
            nc.gpsimd.memset(t[:], 0.0)

        # ------------- diag(mask) ------------------------------------------
        diag_m = sbt.tile([NE, NE], f32, tag="diag_m")
        nc.vector.tensor_scalar(out=diag_m[:], in0=bF("ident", NE),
                                scalar1=bF("mask", NE), scalar2=None, op0=OP.mult)

        pu_cm = tc.tile_pool(name="pu", bufs=3, space="PSUM")
        pu = pu_cm.__enter__()
        pw_cm = tc.tile_pool(name="pw", bufs=1, space="PSUM")
        pw = pw_cm.__enter__()

        # ------------- transposes (masked raw entities) --------------------
        p_tT = pu.tile([128, KD, NE], f32, tag="pu")
        for k in range(KD):
            nc.tensor.transpose(_r(p_tT[:, k, :]), _r(t_ent[:, k * 128:(k + 1) * 128]),
                                _r(diag_m[:]))
        nc.vector.tensor_copy(entT[:], p_tT[:])

        # ------------- gram + cosine ---------------------------------------
        p_g = pu.tile([NE, NE], f32, tag="pu")
        for k in range(KD):
            nc.tensor.matmul(p_g[:], entT[:, k, :], entT[:, k, :],
                             start=(k == 0), stop=(k == KD - 1))
        nc.vector.tensor_copy(gram[:], p_g[:])
        # norms off the diagonal: ss = sum(gram * I)
        dd = sbt.tile([NE, NE], f32, tag="dd")
        nc.vector.tensor_mul(dd[:], gram[:], bF("ident", NE))
        ss = sbt.tile([NE, 1], f32, tag="ss")
        nc.vector.reduce_sum(ss[:], dd[:], axis=mybir.AxisListType.X)
        nrmv = sbt.tile([NE, 1], f32, tag="nrmv")
        nc.scalar.sqrt(nrmv[:], ss[:])
        sgd = sbt.tile([NE, 1], f32, tag="sgd")
        nc.scalar.activation(sgd[:], ss[:], AF.Sigmoid)
        nc.vector.tensor_single_scalar(nrmv[:], nrmv[:], 1e-13, op=OP.max)
        rinv = sbt.tile([NE, 1], f32, tag="rinv")
        nc.vector.reciprocal(rinv[:], nrmv[:])
        nc.vector.tensor_tensor(out=rinv[:], in0=rinv[:], in1=bF("mask", NE),
                                op=OP.mult)
        diag_r = sbt.tile([NE, NE], f32, tag="diag_r")
        nc.vector.tensor_scalar(out=diag_r[:], in0=bF("ident", NE),
                                scalar1=rinv[:], scalar2=None, op0=OP.mult)
        # cos = D * (gram^T * D): transpose-with-diag then row scale
        p_ct = pu.tile([NE, NE], f32, tag="pu")
        nc.tensor.transpose(_r(p_ct[:]), _r(gram[:]), _r(diag_r[:]))
        nc.vector.tensor_scalar(out=s_cos[:], in0=p_ct[:],
                                scalar1=rinv[:], scalar2=None, op0=OP.mult)

        # IC3[dx, r, c] = cos[r-1, c+dx-2] (zero padded)
        for dx, q in ((0, nc.scalar), (1, nc.scalar), (2, nc.gpsimd)):
            c_lo = max(0, 2 - dx)
            c_hi = min(34, 34 - dx)
            ncol = c_hi - c_lo
            s_lo = c_lo + dx - 2
            q.dma_start(IC3[dx:dx + 1, 1:33, c_lo:c_lo + ncol],
                        s_cos[:, s_lo:s_lo + ncol])

        # ------------- premultiply ew = ent_masked @ W1 --------------------
        p_ew = pw.tile([NE, D], f32, tag="pw")

        def premult(ws, kc):
            for n0, n1 in ((0, 512), (512, 768)):
                nc.tensor.matmul(p_ew[:, n0:n1], entT[:, kc, :],
                                 ws[kc][:, n0:n1],
                                 start=(kc == 0), stop=(kc == KD - 1),
                                 skip_group_check=True)

        premult(w1h_s, 0)
        premult(w1h_s, 1)

        # ------------- enc1: 3 row-tap matmuls x 2 N-halves ----------------
        p_c1 = pu.tile([64, 1024], f32, tag="pu")
        for hh in range(2):
            for dy in range(3):
                rr = slice(hh * 16 + dy, hh * 16 + dy + 16)
                nc.tensor.matmul(p_c1[:, hh * 512:(hh + 1) * 512],
                                 bA("enc1", 3)[:, dy * 64:(dy + 1) * 64],
                                 _r(IC3[:, rr, 1:33]),
                                 start=(dy == 0), stop=(dy == 2),
                                 skip_group_check=True)
        nc.scalar.activation(_r(c1p[:]),
                             p_c1[:].rearrange("c (h w) -> c h w", h=32, w=32),
                             AF.Relu, bias=bF("enc1b", 64))

        premult(w1h_s, 2)
        premult(w1h_s, 3)

        # ------------- pool1 -> p1p interior [64, 16, 16] ------------------
        tmp = sbt.tile([64, 16, 16], f32, tag="t")
        nc.vector.tensor_max(tmp[:], c1p[:, 0:32:2, 0:32:2], c1p[:, 0:32:2, 1:32:2])
        nc.vector.tensor_max(tmp[:], tmp[:], c1p[:, 1:32:2, 0:32:2])
        nc.vector.tensor_max(_r(p1p[:, 1:17, 1:17]), tmp[:], c1p[:, 1:32:2, 1:32:2])

        # ------------- enc2: 9 shifted matmuls K=64 ------------------------
        p_c2 = pu.tile([128, 256], f32, tag="pu")
        e2w = bA("enc2", 64).rearrange("c (t m) -> c t m", t=9)
        for tap in range(9):
            dy, dx = tap // 3, tap % 3
            nc.tensor.matmul(p_c2[:], e2w[:, tap, :],
                             _r(p1p[:, dy:dy + 16, dx:dx + 16]),
                             start=(tap == 0), stop=(tap == 8))
        nc.scalar.activation(_r(c2p[:]),
                             p_c2[:].rearrange("c (h w) -> c h w", h=16, w=16),
                             AF.Relu, bias=bF("enc2b"))

        premult(w1h_s, 4)
        premult(w1h_s, 5)
        nc.scalar.activation(ew1[:], p_ew[:], AF.Copy)

        # one-hots for the pair gather of ew rows
        for (src_c, dst) in (("hi_f", ohhi), ("ti_f", ohti)):
            c0, n = FC_COLS[src_c]
            bc = sbt.tile([NE, NH], f32, tag="bc")
            nc.gpsimd.partition_broadcast(bc[:], t_fc[0:1, c0:c0 + n])
            nc.vector.tensor_scalar(out=_r(dst[:]), in0=bc[:],
                                    scalar1=bF("iota", NE), scalar2=None,
                                    op0=OP.is_equal)

        # ------------- pool2 -> p2p interior [128, 8, 8] -------------------
        tmp2 = sbt.tile([128, 8, 8], f32, tag="t")
        nc.vector.tensor_max(tmp2[:], c2p[:, 0:16:2, 0:16:2], c2p[:, 0:16:2, 1:16:2])
        nc.vector.tensor_max(tmp2[:], tmp2[:], c2p[:, 1:16:2, 0:16:2])
        nc.vector.tensor_max(p2p[:, 1:9, 1:9], tmp2[:], c2p[:, 1:16:2, 1:16:2])

        # ------------- bottleneck: 9 taps x 2 M-chunks, K=128 --------------
        bw = bA("bott").rearrange("c (t m) -> c t m", t=9)
        for mc, dst in ((0, c3a), (1, c3b)):
            p_c3 = pu.tile([128, 64], f32, tag="pu")
            for tap in range(9):
                dy, dx = tap // 3, tap % 3
                nc.tensor.matmul(p_c3[:], bw[:, tap, mc * 128:(mc + 1) * 128],
                                 p2p[:, dy:dy + 8, dx:dx + 8],
                                 start=(tap == 0), stop=(tap == 8))
            nc.scalar.activation(dst[:], p_c3[:].rearrange("c (h w) -> c h w", h=8, w=8),
                                 AF.Relu, bias=bF("bottb")[:, mc:mc + 1])

        # ------------- up2 -> u2p interiors --------------------------------
        for src, dst in ((c3a, u2p0), (c3b, u2p1)):
            for i in range(2):
                for j in range(2):
                    nc.vector.tensor_copy(_r(dst[:, 1 + i:17:2, 1 + j:17:2]), src[:])

        # ------------- attention gate 2 (pre-upsample trick) ---------------
        wg2 = bA("ag2wg").rearrange("c (t m) -> c t m", t=2)
        p_q2 = pu.tile([128, 8, 8], f32, tag="pu")
        nc.tensor.matmul(p_q2[:], wg2[:, 0, :], c3a[:], start=True, stop=False)
        nc.tensor.matmul(p_q2[:], wg2[:, 1, :], c3b[:], start=False, stop=True)
        p_x2 = pu.tile([128, 16, 16], f32, tag="pu")
        nc.tensor.matmul(p_x2[:], bA("ag2wx"), c2p[:].bitcast(f32r))
        r2 = sbt.tile([128, 16, 16], f32, tag="r2")
        q2b = p_q2[:].unsqueeze(2).unsqueeze(4).broadcast_to([128, 8, 2, 8, 2])
        nc.vector.tensor_tensor(out=r2[:].rearrange("c (h a) (w b) -> c h a w b", a=2, b=2),
                                in0=p_x2[:].rearrange("c (h a) (w b) -> c h a w b", a=2, b=2),
                                in1=q2b, op=OP.add)
        nc.vector.tensor_single_scalar(r2[:], r2[:], 0.0, op=OP.max)
        p_g2 = pu.tile([1, 256], f32, tag="pu")
        nc.tensor.matmul(p_g2[:], bA("ag2psi"), _r(r2[:].rearrange("c h w -> c (h w)")))
        a2 = sbt.tile([1, 256], f32, tag="a2")
        nc.scalar.activation(_r(a2[:]), p_g2[:], AF.Sigmoid)
        p_a2b = pu.tile([128, 256], f32, tag="pu")
        nc.tensor.matmul(p_a2b[:], _r(bF("ones", 1)), _r(a2[:]))
        nc.vector.tensor_mul(_r(att2p[:, 1:17, 1:17]),
                             p_a2b[:].rearrange("c (h w) -> c h w", h=16, w=16), c2p[:])

        # ------------- dec2: 9 taps x 3 K-chunks ---------------------------
        p_d2 = pu.tile([128, 256], f32, tag="pu")
        d2w = bB("dec2").rearrange("c (s t m) -> c s t m", s=3, t=9)
        srcs2 = (u2p0, u2p1, att2p)
        n_mm = 0
        for tap in range(9):
            dy, dx = tap // 3, tap % 3
            for kc in range(3):
                nc.tensor.matmul(p_d2[:], d2w[:, kc, tap, :],
                                 _r(srcs2[kc][:, dy:dy + 16, dx:dx + 16]),
                                 start=(n_mm == 0), stop=(n_mm == 26))
                n_mm += 1
        nc.scalar.activation(_r(d2pad[:, 1:17, 1:17]),
                             p_d2[:].rearrange("c (h w) -> c h w", h=16, w=16),
                             AF.Relu, bias=bF("dec2b"))

        # ------------- attention gate 1 (pre-upsample trick) ---------------
        p_q1 = pu.tile([64, 16, 16], f32, tag="pu")
        nc.tensor.matmul(p_q1[:], bB("ag1wg"), _r(d2pad[:, 1:17, 1:17]))
        p_x1 = pu.tile([64, 1024], f32, tag="pu")
        c1v = c1p[:].rearrange("c h w -> c (h w)")
        for hh in range(2):
            nc.tensor.matmul(p_x1[:, hh * 512:(hh + 1) * 512], bB("ag1wx", 64),
                             _r(c1v[:, hh * 512:(hh + 1) * 512]),
                             start=True, stop=True, skip_group_check=True)
        r1a = sbt.tile([64, 32, 32], f32, tag="r1a")
        q1b = p_q1[:].unsqueeze(2).unsqueeze(4).broadcast_to([64, 16, 2, 16, 2])
        nc.vector.tensor_tensor(out=r1a[:].rearrange("c (h a) (w b) -> c h a w b", a=2, b=2),
                                in0=p_x1[:].rearrange("c (h a w b) -> c h a w b", h=16, a=2, w=16, b=2),
                                in1=q1b, op=OP.add)
        r1 = sbt.tile([64, 32, 32], f32, tag="r1")
        nc.scalar.activation(r1[:], r1a[:], AF.Relu)
        r1v = r1[:].rearrange("c h w -> c (h w)")
        a1 = sbt.tile([1, 1024], f32, tag="a1")
        for hh in range(2):
            p_g1 = pu.tile([1, 512], f32, tag="pu")
            nc.tensor.matmul(p_g1[:], bB("ag1psi", 64),
                             _r(r1v[:, hh * 512:(hh + 1) * 512]))
            nc.scalar.activation(_r(a1[:, hh * 512:(hh + 1) * 512]), p_g1[:], AF.Sigmoid)
            p_a1b = pu.tile([64, 512], f32, tag="pu")
            nc.tensor.matmul(p_a1b[:], _r(bF("ones", 1)[:, 0:64]),
                             _r(a1[:, hh * 512:(hh + 1) * 512]))
            nc.vector.tensor_mul(
                _r(att1p[:, 1 + 16 * hh:17 + 16 * hh, 1:33]),
                p_a1b[:].rearrange("c (h w) -> c h w", h=16, w=32),
                c1p[:, 16 * hh:16 * hh + 16, :])

        # premult tail (W1t stream lands late UNet)
        premult(w1t_s, 0)
        premult(w1t_s, 1)

        # ------------- dec1: 4-phase (u-part 2x2 cells + att 9 taps) -------
        d1ph = bB("d1ph").rearrange("c (p l m) -> c p l m", p=4, l=4)
        d1at = bB("d1att", 64).rearrange("c (t m) -> c t m", t=9)
        for a in range(2):
            for b in range(2):
                ph_i = a * 2 + b
                p_d1 = pu.tile([64, 16, 16], f32, tag="pu")
                n_mm = 0
                for cu in range(2):
                    for cv in range(2):
                        nc.tensor.matmul(p_d1[:], d1ph[:, ph_i, cu * 2 + cv, :],
                                         _r(d2pad[:, cu + a:cu + a + 16,
                                                  cv + b:cv + b + 16]),
                                         start=(n_mm == 0), stop=False)
                        n_mm += 1
                for tap in range(9):
                    dy, dx = tap // 3, tap % 3
                    nc.tensor.matmul(p_d1[:], d1at[:, tap, :],
                                     _r(att1p[:, a + dy:a + dy + 31:2,
                                              b + dx:b + dx + 31:2]),
                                     start=False, stop=(tap == 8))
                nc.scalar.activation(_r(d1s[:, a:32:2, b:32:2]), p_d1[:],
                                     AF.Relu, bias=bF("dec1b", 64))
            if a == 0:
                premult(w1t_s, 2)
                premult(w1t_s, 3)

        premult(w1t_s, 4)
        premult(w1t_s, 5)
        nc.scalar.activation(et1[:], p_ew[:], AF.Copy)

        # ------------- fin 1x1 conv -> amapT [256, 1024] -------------------
        d1v = d1s[:].rearrange("c h w -> c (h w)")
        for mc, dst in ((0, amap0), (1, amap1)):
            p_am = pu.tile([128, 1024], f32, tag="pu")
            for hh in range(2):
                nc.tensor.matmul(p_am[:, hh * 512:(hh + 1) * 512],
                                 bB("fin", 64)[:, mc * 128:(mc + 1) * 128],
                                 _r(d1v[:, hh * 512:(hh + 1) * 512]),
                                 start=True, stop=True, skip_group_check=True)
            nc.scalar.activation(dst[:], p_am[:], AF.Identity,
                                 bias=bF("finb")[:, mc:mc + 1])

        # amap pair gathers
        c0, n = I16_COLS["pidx"]
        pidx = t_i16[:, c0:c0 + n]
        nc.gpsimd.ap_gather(htT0[:].rearrange("c (n o) -> c n o", o=1),
                            amap0[:].rearrange("c (n o) -> c n o", o=1), pidx,
                            channels=128, num_elems=1024, d=1, num_idxs=NH)
        nc.gpsimd.ap_gather(htT1[:].rearrange("c (n o) -> c n o", o=1),
                            amap1[:].rearrange("c (n o) -> c n o", o=1), pidx,
                            channels=128, num_elems=1024, d=1, num_idxs=NH)

        pw_cm.__exit__(None, None, None)
        pu_cm.__exit__(None, None, None)

        # ------------- pair features + decoder -----------------------------
        ph_cm = tc.tile_pool(name="ph", bufs=4, space="PSUM")
        ph = ph_cm.__enter__()
        pd_cm = tc.tile_pool(name="pd", bufs=2, space="PSUM")
        pd = pd_cm.__enter__()
        po_cm = tc.tile_pool(name="po", bufs=1, space="PSUM")
        po = po_cm.__enter__()
        p_out = po.tile([2, NH], f32, tag="po")
        w2h = bC("w2h").rearrange("c (t m) -> c t m", t=2)
        w2t = bC("w2t").rearrange("c (t m) -> c t m", t=2)
        wde = bC("wdec").rearrange("c (g m) -> c g m", g=G)
        for k in range(KD):
            cols = slice(k * 128, (k + 1) * 128)
            for (w2, ewt, oh, bp, dstT) in ((w2h, ew1, ohhi, "hbp", hsT),
                                            (w2t, et1, ohti, "tbp", tsT)):
                p_hs = ph.tile([128, NH], f32, tag="ph")
                nc.tensor.matmul(p_hs[:], _r(ewt[:, cols]), _r(oh[:]),
                                 start=True, stop=False)
                nc.tensor.matmul(p_hs[:], w2[:, 0, cols], _r(htT0[:]),
                                 start=False, stop=False)
                nc.tensor.matmul(p_hs[:], w2[:, 1, cols], _r(htT1[:]),
                                 start=False, stop=True)
                nc.scalar.activation(dstT[:, k, :], p_hs[:],
                                     AF.Tanh, bias=bF(bp)[:, k:k + 1])
            for half in range(2):
                g = 2 * k + half
                rows = slice(half * 64, (half + 1) * 64)
                p_u = pd.tile([128, NH], f32, tag="pd")
                nc.tensor.matmul(p_u[:], wde[rows, g, :], tsT[rows, k, :])
                pu_sb = sbt.tile([128, NH], bf16, tag="pusb")
                eng = nc.gpsimd if k < 4 else nc.scalar
                if eng is nc.gpsimd:
                    nc.gpsimd.tensor_copy(pu_sb[:], p_u[:])
                else:
                    nc.scalar.activation(pu_sb[:], p_u[:], AF.Copy)
                v = sbt.tile([128, NH], bf16, tag="v")
                nc.vector.tensor_mul(v[0:64, :], pu_sb[0:64, :], hsT[rows, k, :])
                nc.vector.tensor_mul(v[64:128, :], pu_sb[64:128, :], hsT[rows, k, :])
                nc.tensor.matmul(p_out[:], bA("smat"), v[:],
                                 start=(g == 0), stop=(g == G - 1),
                                 skip_group_check=True)
        out_sb = sbt.tile([2, NH], f32, tag="out")
        nc.scalar.activation(out_sb[:], p_out[:], AF.Identity, bias=bF("decb", 2))
        nc.sync.dma_start(y[:], out_sb[:])
        po_cm.__exit__(None, None, None)
        pd_cm.__exit__(None, None, None)
        ph_cm.__exit__(None, None, None)

    nc.compile()
    return nc


def _wrap16(idx, n_slots):
    """int16 index layout for gpsimd gathers: wrapped in 16 partitions,
    replicated across the 8 gpsimd cores."""
    out = np.zeros((128, n_slots), np.int16)
    for j, v in enumerate(idx):
        out[np.arange(8) * 16 + j % 16, j // 16] = v
    return out


def pack_inputs(inputs):
    """Build the 8 per-core input maps from the full problem inputs."""
    import ml_dtypes
    x = np.asarray(inputs["x"], np.float32)
    entity_pos = np.asarray(inputs["entity_pos"])
    hts = np.asarray(inputs["hts"])

    def W(name):
        return np.asarray(inputs[name], np.float32)

    def blob(layout, ncols, parts_map):
        b = np.zeros((128, ncols), np.float32)
        for name, arr in parts_map.items():
            c0, n = layout[name]
            p = arr.shape[0]
            b[0:p, c0:c0 + n] = arr.reshape(p, n)
        return b

    shared = {}
    e1 = W("enc1_w").reshape(64, 9)            # [c, dy*3+dx]
    enc1 = np.zeros((3, 3 * 64), np.float32)   # [dx, dy*64+c]
    for dy in range(3):
        for dx in range(3):
            enc1[dx, dy * 64:(dy + 1) * 64] = e1[:, dy * 3 + dx]
    smat = np.zeros((128, 2), np.float32)
    smat[:64, 0] = 1.0
    smat[64:, 1] = 1.0
    blobA = blob(BA_COLS, CA, {
        "enc1": enc1,
        "enc2": W("enc2_w").reshape(128, 64, 9).transpose(1, 2, 0).copy(),
        "bott": W("bott_w").reshape(256, 128, 9).transpose(1, 2, 0).copy(),
        "ag2wg": W("ag2_wg").reshape(128, 256).T.reshape(2, 128, 128).transpose(1, 0, 2).copy(),
        "ag2wx": W("ag2_wx").reshape(128, 128).T.copy(),
        "ag2psi": W("ag2_psi").reshape(1, 128).T.copy(),
        "smat": smat,
    })
    d1w = W("dec1_w")                          # [64, 192, 3, 3]
    du = d1w[:, 0:128]                         # u-part [64, 128, 3, 3]
    d1ph = np.zeros((128, 4, 4, 64), np.float32)
    taps_u = {(0, 0): [0], (0, 1): [1, 2], (1, 0): [0, 1], (1, 1): [2]}
    for a in range(2):
        for b_ in range(2):
            for cu in range(2):
                for cv in range(2):
                    acc = np.zeros((128, 64), np.float32)
                    for dy in taps_u[(a, cu)]:
                        for dx in taps_u[(b_, cv)]:
                            acc += du[:, :, dy, dx].T
                    d1ph[:, a * 2 + b_, cu * 2 + cv, :] = acc
    blobB = blob(BB_COLS, CB, {
        "dec2": W("dec2_w").reshape(128, 384, 9).transpose(1, 2, 0)
                .reshape(3, 128, 9, 128).transpose(1, 0, 2, 3).copy(),
        "ag1wg": W("ag1_wg").reshape(64, 128).T.copy(),
        "ag1wx": W("ag1_wx").reshape(64, 64).T.copy(),
        "ag1psi": W("ag1_psi").reshape(1, 64).T.copy(),
        "d1ph": d1ph,
        "d1att": d1w[:, 128:192].reshape(64, 64, 9).transpose(1, 2, 0).copy(),
        "fin": W("fin_w").reshape(256, 64).T.copy(),
    })
    head_w = W("head_w"); tail_w = W("tail_w")
    wd = W("decoder_w").reshape(G, 64, 64, 2).transpose(2, 0, 3, 1).reshape(64, G, 128)
    blobC = blob(BC_COLS, CC, {
        "w2h": head_w[D:].reshape(2, 128, D).transpose(1, 0, 2).copy(),
        "w2t": tail_w[D:].reshape(2, 128, D).transpose(1, 0, 2).copy(),
        "wdec": np.concatenate([wd, wd], axis=0).copy(),
    })
    for k, v in (("blobA", blobA), ("blobB", blobB), ("blobC", blobC)):
        shared[k] = v.astype(ml_dtypes.bfloat16)
    shared["W1h"] = head_w[:D].reshape(KD, 128, D).transpose(1, 0, 2).astype(ml_dtypes.bfloat16)
    shared["W1t"] = tail_w[:D].reshape(KD, 128, D).transpose(1, 0, 2).astype(ml_dtypes.bfloat16)

    f32_shared = {
        "ident": np.eye(NE, dtype=np.float32),
        "enc1b": W("enc1_b").reshape(64, 1),
        "enc2b": W("enc2_b").reshape(128, 1),
        "bottb": W("bott_b").reshape(2, 128).T.copy(),
        "dec2b": W("dec2_b").reshape(128, 1),
        "dec1b": W("dec1_b").reshape(64, 1),
        "finb": W("fin_b").reshape(2, 128).T.copy(),
        "hbp": W("head_b").reshape(KD, 128).T.copy(),
        "tbp": W("tail_b").reshape(KD, 128).T.copy(),
        "decb": W("decoder_b").reshape(2, 1),
        "iota": np.arange(NE, dtype=np.float32).reshape(NE, 1),
        "ones": np.ones((1, 128), np.float32),
    }

    in_maps = []
    for c in range(NCORES):
        b, h = c // 2, c % 2
        m = dict(shared)
        start = entity_pos[b, :, 0].astype(np.int64)
        idx = np.minimum(start + 1, L - 1)
        mask = (start + 1 < L).astype(np.float32).reshape(NE, 1)
        m["ent_in"] = np.ascontiguousarray(x[b][idx])
        fm = dict(f32_shared)
        fm["mask"] = mask
        f32bl = np.zeros((128, CF), np.float32)
        for name, arr in fm.items():
            c0, n = F32_COLS[name]
            p = arr.shape[0]
            f32bl[0:p, c0:c0 + n] = arr.reshape(p, n)
        m["f32b"] = f32bl
        hi = hts[b, h * NH:(h + 1) * NH, 0].astype(np.int64)
        ti = hts[b, h * NH:(h + 1) * NH, 1].astype(np.int64)
        fcb = np.zeros((1, CFC), np.float32)
        fcb[0, FC_COLS["hi_f"][0]:FC_COLS["hi_f"][0] + NH] = hi
        fcb[0, FC_COLS["ti_f"][0]:FC_COLS["ti_f"][0] + NH] = ti
        m["f32c"] = fcb
        i16bl = np.zeros((128, CI), np.int16)
        c0, n = I16_COLS["pidx"]
        i16bl[:, c0:c0 + n] = _wrap16((hi * NE + ti).astype(np.int16), NH // 16)
        m["i16b"] = i16bl
        in_maps.append(m)
    return in_maps


_NC_CACHE = None


def get_nc():
    global _NC_CACHE
    if _NC_CACHE is None:
        _NC_CACHE = build_nc()
    return _NC_CACHE


def kernel(**inputs):
    nc = get_nc()
    in_maps = pack_inputs(inputs)
    res = run_bass_kernel_spmd(nc, in_maps, core_ids=list(range(NCORES)))
    out = np.empty((B * P, 2), np.float32)
    for c in range(NCORES):
        b, h = c // 2, c % 2
        yc = res.results[c]["y"]                  # [2, NH]
        out[b * P + h * NH:b * P + (h + 1) * NH, :] = yc.T
    return out


# revision 17
# speedup vs baseline: 1.6552x; 1.1296x over previous
"""Trainium2 Bass kernel for nn_CoreferenceResolver (coref UNet + pair decoder).

Sharding: core c handles batch b=c//2 and pair-half h=c%2 (496 of 992 pairs).
The gather/cosine/UNet stages are replicated on the two cores sharing a batch;
the extractor linears and group-bilinear decoder are sharded over pairs.

Weights are packed host-side into bf16 blobs + two streamed bf16 W1 tensors so
the whole weight set moves in ~20 DMAs. The cosine matrix is computed as a
masked gram matrix (entity norms come off its diagonal), enc1 runs on a
3-partition column im2col, dec1 uses a 4-phase decomposition of the upsample
conv, and the attention gates apply their 1x1 convs before upsampling.
"""
import os
import sys

for _p in ("/opt/trn_rl_repo",):
    if os.path.isdir(_p) and _p not in sys.path:
        sys.path.insert(0, _p)

import numpy as np

import concourse.bass as bass
import concourse.tile as tile
from concourse import bacc, mybir
from concourse.bass_utils import run_bass_kernel_spmd

f32 = mybir.dt.float32
i16 = mybir.dt.int16
AF = mybir.ActivationFunctionType
OP = mybir.AluOpType
f32r = mybir.dt.float32r
bf16 = mybir.dt.bfloat16


def _r(ap):
    """View an fp32 AP as float32r for full-rate PE streaming."""
    return ap.bitcast(f32r)

B, L, D, H = 4, 1024, 768, 12
NE, P = 32, 992
BLOCK = 64
G = D // BLOCK          # 12 groups
OUT_CH = 256
NCORES = 8
NH = P // 2             # 496 pairs per core
KD = D // 128           # 6 chunks of the D dim


def _mk_layout(entries):
    cols = {}
    c = 0
    for name, n in entries:
        cols[name] = (c, n)
        c += n
    return cols, c

BA_COLS, CA = _mk_layout([
    ("enc1", 3 * 64),       # [3(dx), 3(dy)*64]
    ("enc2", 9 * 128),      # [64, 9, 128]
    ("bott", 9 * 256),      # [128, 9, 256]
    ("ag2wg", 2 * 128),     # [128, 2, 128]
    ("ag2wx", 128),         # [128, 128]
    ("ag2psi", 1),          # [128, 1]
    ("smat", 2),            # [128, 2]
])

BB_COLS, CB = _mk_layout([
    ("dec2", 3 * 9 * 128),  # [128, 3, 9, 128]
])

BB2_COLS, CB2 = _mk_layout([
    ("ag1wg", 64),          # [128, 64]
    ("ag1wx", 64),          # [64, 64]
    ("ag1psi", 1),          # [64, 1]
    ("d1ph", 4 * 4 * 64),   # [128, 4(phase), 4(cell), 64]
    ("d1att", 9 * 64),      # [64, 9, 64]
    ("fin", 256),           # [64, 256]
])

BC_COLS, CC = _mk_layout([
    ("w2h", 2 * 768),       # [128, 2, 768]
])

BC2_COLS, CC2 = _mk_layout([
    ("w2t", 2 * 768),       # [128, 2, 768]
    ("wdec", G * 128),      # [128, G, 128]
])

F32_COLS, CF = _mk_layout([
    ("ident", NE),          # [32, 32]
    ("mask", 1),            # [32, 1]
    ("enc1b", 1),           # [64, 1]
    ("enc2b", 1),           # [128, 1]
    ("bottb", 2),           # [128, 2]
    ("dec2b", 1),           # [128, 1]
    ("dec1b", 1),           # [64, 1]
    ("finb", 2),            # [128, 2]
    ("hbp", KD),            # [128, KD]
    ("tbp", KD),            # [128, KD]
    ("decb", 1),            # [2, 1]
    ("iota", 1),            # [32, 1]
    ("ones", 128),          # [1, 128]
])

FC_COLS, CFC = _mk_layout([
    ("hi_f", NH),           # [1, NH]
    ("ti_f", NH),           # [1, NH]
])

I16_COLS, CI = _mk_layout([
    ("pidx", NH // 16),     # amap pair gather
])


def build_nc():
    nc = bacc.Bacc("TRN2", target_bir_lowering=False, debug=False, num_devices=NCORES)

    def inp(name, shape, dt=f32):
        return nc.dram_tensor(name, shape, dt, kind="ExternalInput")

    ent_in = inp("ent_in", [NE, D])
    i16b  = inp("i16b", [128, CI], i16)
    f32b  = inp("f32b", [128, CF])
    f32c  = inp("f32c", [1, CFC])
    blobA = inp("blobA", [128, CA], bf16)
    blobB = inp("blobB", [128, CB], bf16)
    blobB2 = inp("blobB2", [128, CB2], bf16)
    blobC = inp("blobC", [128, CC], bf16)
    blobC2 = inp("blobC2", [128, CC2], bf16)
    W1h   = inp("W1h", [128, KD, D], bf16)
    W1t   = inp("W1t", [128, KD, D], bf16)

    y = nc.dram_tensor("y", [2, NH], f32, kind="ExternalOutput")

    from contextlib import ExitStack
    with tile.TileContext(nc) as tc, ExitStack() as _ctx:
        sbw = _ctx.enter_context(tc.tile_pool(name="sbw", bufs=1))   # persistent
        sbt = _ctx.enter_context(tc.tile_pool(name="sbt", bufs=3))   # rotating temps

        # ------------- DMA issue: SP queue in schedule order ---------------
        t_ent = sbw.tile([NE, D], f32, tag="ent")
        nc.sync.dma_start(t_ent[:], ent_in[:])
        t_i16 = sbw.tile([128, CI], i16, tag="i16")
        nc.sync.dma_start(t_i16[:], i16b[:])
        t_f32 = sbw.tile([128, CF], f32, tag="f32")
        nc.sync.dma_start(t_f32[:], f32b[:])
        t_bA = sbw.tile([128, CA], bf16, tag="bA")
        nc.sync.dma_start(t_bA[:], blobA[:])
        t_fc = sbw.tile([1, CFC], f32, tag="fc")
        nc.sync.dma_start(t_fc[:], f32c[:])
        w1h_s = []
        for k in range(KD):
            t = sbw.tile([128, D], bf16, tag=f"w1h{k}")
            nc.sync.dma_start(t[:], W1h[:, k, :])
            w1h_s.append(t)
        w1t_s = []
        for k in range(KD):
            t = sbw.tile([128, D], bf16, tag=f"w1t{k}")
            nc.sync.dma_start(t[:], W1t[:, k, :])
            w1t_s.append(t)
        t_bB = sbw.tile([128, CB], bf16, tag="bB")
        nc.sync.dma_start(t_bB[:], blobB[:])
        t_bB2 = sbw.tile([128, CB2], bf16, tag="bB2")
        nc.sync.dma_start(t_bB2[:], blobB2[:])
        t_bC = sbw.tile([128, CC], bf16, tag="bC")
        nc.sync.dma_start(t_bC[:], blobC[:])
        t_bC2 = sbw.tile([128, CC2], bf16, tag="bC2")
        nc.sync.dma_start(t_bC2[:], blobC2[:])

        def bA(name, parts=128):
            c0, n = BA_COLS[name]
            return t_bA[0:parts, c0:c0 + n]

        def bB(name, parts=128):
            if name in BB_COLS:
                c0, n = BB_COLS[name]
                return t_bB[0:parts, c0:c0 + n]
            c0, n = BB2_COLS[name]
            return t_bB2[0:parts, c0:c0 + n]

        def bC(name, parts=128):
            if name in BC_COLS:
                c0, n = BC_COLS[name]
                return t_bC[0:parts, c0:c0 + n]
            c0, n = BC2_COLS[name]
            return t_bC2[0:parts, c0:c0 + n]

        def bF(name, parts=128):
            c0, n = F32_COLS[name]
            return t_f32[0:parts, c0:c0 + n]

        # ------------- persistent SBUF intermediates -----------------------
        entT  = sbw.tile([128, KD, NE], bf16, tag="entT")
        gram  = sbw.tile([NE, NE], f32, tag="gram")
        s_cos = sbw.tile([NE, NE], f32, tag="scos")
        IC3   = sbw.tile([3, 34, 34], f32, tag="IC3")
        c1p   = sbw.tile([64, 32, 32], f32, tag="c1p")     # dense
        p1p   = sbw.tile([64, 18, 18], f32, tag="p1p")     # padded
        c2p   = sbw.tile([128, 16, 16], f32, tag="c2p")    # dense
        p2p   = sbw.tile([128, 10, 10], bf16, tag="p2p")   # padded
        c3a   = sbw.tile([128, 8, 8], bf16, tag="c3a")     # dense
        c3b   = sbw.tile([128, 8, 8], bf16, tag="c3b")
        u2p0  = sbw.tile([128, 18, 18], f32, tag="u2p0")   # padded
        u2p1  = sbw.tile([128, 18, 18], f32, tag="u2p1")
        att2p = sbw.tile([128, 18, 18], f32, tag="att2p")
        d2pad = sbw.tile([128, 18, 18], f32, tag="d2pad")
        att1p = sbw.tile([64, 34, 34], f32, tag="att1p")
        d1s   = sbw.tile([64, 32, 32], f32, tag="d1s")     # dense
        amap0 = sbw.tile([128, 1024], f32, tag="amap0")
        amap1 = sbw.tile([128, 1024], f32, tag="amap1")
        ew1   = sbw.tile([NE, D], f32, tag="ew1")
        et1   = sbw.tile([NE, D], f32, tag="et1")
        ohhi  = sbw.tile([NE, NH], f32, tag="ohhi")
        ohti  = sbw.tile([NE, NH], f32, tag="ohti")
        htT0  = sbw.tile([128, NH], f32, tag="htT0")
        htT1  = sbw.tile([128, NH], f32, tag="htT1")
        hsT   = sbw.tile([128, KD, NH], bf16, tag="hsT")
        tsT   = sbw.tile([128, KD, NH], bf16, tag="tsT")

        # ------------- Pool queue: memsets, later broadcasts/gathers -------
        for t in (IC3, p1p, p2p, u2p0, u2p1, att2p, d2pad, att1p):
            nc.gpsimd.memset(t[:], 0.0)

        # ------------- diag(mask) ------------------------------------------
        diag_m = sbt.tile([NE, NE], f32, tag="diag_m")
        nc.vector.tensor_scalar(out=diag_m[:], in0=bF("ident", NE),
                                scalar1=bF("mask", NE), scalar2=None, op0=OP.mult)

        pu_cm = tc.tile_pool(name="pu", bufs=3, space="PSUM")
        pu = pu_cm.__enter__()
        pw_cm = tc.tile_pool(name="pw", bufs=1, space="PSUM")
        pw = pw_cm.__enter__()

        # ------------- transposes (masked raw entities) --------------------
        p_tT = pu.tile([128, KD, NE], f32, tag="pu")
        for k in range(KD):
            nc.tensor.transpose(_r(p_tT[:, k, :]), _r(t_ent[:, k * 128:(k + 1) * 128]),
                                _r(diag_m[:]))
        nc.vector.tensor_copy(entT[:], p_tT[:])

        # ------------- gram + cosine ---------------------------------------
        p_g = pu.tile([NE, NE], f32, tag="pu")
        for k in range(KD):
            nc.tensor.matmul(p_g[:], entT[:, k, :], entT[:, k, :],
                             start=(k == 0), stop=(k == KD - 1))
        nc.vector.tensor_copy(gram[:], p_g[:])
        # norms off the diagonal: ss = sum(gram * I)
        dd = sbt.tile([NE, NE], f32, tag="dd")
        nc.vector.tensor_mul(dd[:], gram[:], bF("ident", NE))
        ss = sbt.tile([NE, 1], f32, tag="ss")
        nc.vector.reduce_sum(ss[:], dd[:], axis=mybir.AxisListType.X)
        nrmv = sbt.tile([NE, 1], f32, tag="nrmv")
        nc.scalar.sqrt(nrmv[:], ss[:])
        sgd = sbt.tile([NE, 1], f32, tag="sgd")
        nc.scalar.activation(sgd[:], ss[:], AF.Sigmoid)
        nc.vector.tensor_single_scalar(nrmv[:], nrmv[:], 1e-13, op=OP.max)
        rinv = sbt.tile([NE, 1], f32, tag="rinv")
        nc.vector.reciprocal(rinv[:], nrmv[:])
        nc.vector.tensor_tensor(out=rinv[:], in0=rinv[:], in1=bF("mask", NE),
                                op=OP.mult)
        diag_r = sbt.tile([NE, NE], f32, tag="diag_r")
        nc.vector.tensor_scalar(out=diag_r[:], in0=bF("ident", NE),
                                scalar1=rinv[:], scalar2=None, op0=OP.mult)
        # cos = D * (gram^T * D): transpose-with-diag then row scale
        p_ct = pu.tile([NE, NE], f32, tag="pu")
        nc.tensor.transpose(_r(p_ct[:]), _r(gram[:]), _r(diag_r[:]))
        nc.vector.tensor_scalar(out=s_cos[:], in0=p_ct[:],
                                scalar1=rinv[:], scalar2=None, op0=OP.mult)

        # IC3[dx, r, c] = cos[r-1, c+dx-2] (zero padded)
        for dx, q in ((0, nc.scalar), (1, nc.scalar), (2, nc.gpsimd)):
            c_lo = max(0, 2 - dx)
            c_hi = min(34, 34 - dx)
            ncol = c_hi - c_lo
            s_lo = c_lo + dx - 2
            q.dma_start(IC3[dx:dx + 1, 1:33, c_lo:c_lo + ncol],
                        s_cos[:, s_lo:s_lo + ncol])

        # ------------- premultiply ew = ent_masked @ W1 --------------------
        p_ew = pw.tile([NE, D], f32, tag="pw")

        def premult(ws, kc):
            for n0, n1 in ((0, 512), (512, 768)):
                nc.tensor.matmul(p_ew[:, n0:n1], entT[:, kc, :],
                                 ws[kc][:, n0:n1],
                                 start=(kc == 0), stop=(kc == KD - 1),
                                 skip_group_check=True)

        premult(w1h_s, 0)
        premult(w1h_s, 1)

        # ------------- enc1: 3 row-tap matmuls x 2 N-halves ----------------
        for hh in range(2):
            p_c1 = pu.tile([64, 512], f32, tag="pu")
            for dy in range(3):
                rr = slice(hh * 16 + dy, hh * 16 + dy + 16)
                nc.tensor.matmul(p_c1[:],
                                 bA("enc1", 3)[:, dy * 64:(dy + 1) * 64],
                                 _r(IC3[:, rr, 1:33]),
                                 start=(dy == 0), stop=(dy == 2))
            nc.scalar.activation(_r(c1p[:, hh * 16:hh * 16 + 16, :]),
                                 p_c1[:].rearrange("c (h w) -> c h w", h=16, w=32),
                                 AF.Relu, bias=bF("enc1b", 64))

        premult(w1h_s, 2)
        premult(w1h_s, 3)

        # ------------- pool1 -> p1p interior [64, 16, 16] ------------------
        tmp = sbt.tile([64, 16, 16], f32, tag="t")
        nc.vector.tensor_max(tmp[:], c1p[:, 0:32:2, 0:32:2], c1p[:, 0:32:2, 1:32:2])
        nc.vector.tensor_max(tmp[:], tmp[:], c1p[:, 1:32:2, 0:32:2])
        nc.vector.tensor_max(_r(p1p[:, 1:17, 1:17]), tmp[:], c1p[:, 1:32:2, 1:32:2])

        # ------------- enc2: 9 shifted matmuls K=64 ------------------------
        p_c2 = pu.tile([128, 256], f32, tag="pu")
        e2w = bA("enc2", 64).rearrange("c (t m) -> c t m", t=9)
        for tap in range(9):
            dy, dx = tap // 3, tap % 3
            nc.tensor.matmul(p_c2[:], e2w[:, tap, :],
                             _r(p1p[:, dy:dy + 16, dx:dx + 16]),
                             start=(tap == 0), stop=(tap == 8))
        nc.scalar.activation(_r(c2p[:]),
                             p_c2[:].rearrange("c (h w) -> c h w", h=16, w=16),
                             AF.Relu, bias=bF("enc2b"))

        premult(w1h_s, 4)
        premult(w1h_s, 5)
        nc.scalar.activation(ew1[:], p_ew[:], AF.Copy)

        # one-hots for the pair gather of ew rows
        for (src_c, dst) in (("hi_f", ohhi), ("ti_f", ohti)):
            c0, n = FC_COLS[src_c]
            bc = sbt.tile([NE, NH], f32, tag="bc")
            nc.gpsimd.partition_broadcast(bc[:], t_fc[0:1, c0:c0 + n])
            nc.vector.tensor_scalar(out=_r(dst[:]), in0=bc[:],
                                    scalar1=bF("iota", NE), scalar2=None,
                                    op0=OP.is_equal)

        # ------------- pool2 -> p2p interior [128, 8, 8] -------------------
        tmp2 = sbt.tile([128, 8, 8], f32, tag="t")
        nc.vector.tensor_max(tmp2[:], c2p[:, 0:16:2, 0:16:2], c2p[:, 0:16:2, 1:16:2])
        nc.vector.tensor_max(tmp2[:], tmp2[:], c2p[:, 1:16:2, 0:16:2])
        nc.vector.tensor_max(p2p[:, 1:9, 1:9], tmp2[:], c2p[:, 1:16:2, 1:16:2])

        # ------------- bottleneck: 9 taps x 2 M-chunks, K=128 --------------
        bw = bA("bott").rearrange("c (t m) -> c t m", t=9)
        for mc, dst in ((0, c3a), (1, c3b)):
            p_c3 = pu.tile([128, 64], f32, tag="pu")
            for tap in range(9):
                dy, dx = tap // 3, tap % 3
                nc.tensor.matmul(p_c3[:], bw[:, tap, mc * 128:(mc + 1) * 128],
                                 p2p[:, dy:dy + 8, dx:dx + 8],
                                 start=(tap == 0), stop=(tap == 8))
            nc.scalar.activation(dst[:], p_c3[:].rearrange("c (h w) -> c h w", h=8, w=8),
                                 AF.Relu, bias=bF("bottb")[:, mc:mc + 1])

        # ------------- up2 -> u2p interiors --------------------------------
        for src, dst in ((c3a, u2p0), (c3b, u2p1)):
            for i in range(2):
                for j in range(2):
                    nc.vector.tensor_copy(_r(dst[:, 1 + i:17:2, 1 + j:17:2]), src[:])

        # ------------- attention gate 2 (pre-upsample trick) ---------------
        wg2 = bA("ag2wg").rearrange("c (t m) -> c t m", t=2)
        p_q2 = pu.tile([128, 8, 8], f32, tag="pu")
        nc.tensor.matmul(p_q2[:], wg2[:, 0, :], c3a[:], start=True, stop=False)
        nc.tensor.matmul(p_q2[:], wg2[:, 1, :], c3b[:], start=False, stop=True)
        p_x2 = pu.tile([128, 16, 16], f32, tag="pu")
        nc.tensor.matmul(p_x2[:], bA("ag2wx"), c2p[:].bitcast(f32r))
        r2 = sbt.tile([128, 16, 16], f32, tag="r2")
        q2b = p_q2[:].unsqueeze(2).unsqueeze(4).broadcast_to([128, 8, 2, 8, 2])
        nc.vector.tensor_tensor(out=r2[:].rearrange("c (h a) (w b) -> c h a w b", a=2, b=2),
                                in0=p_x2[:].rearrange("c (h a) (w b) -> c h a w b", a=2, b=2),
                                in1=q2b, op=OP.add)
        nc.vector.tensor_single_scalar(r2[:], r2[:], 0.0, op=OP.max)
        p_g2 = pu.tile([1, 256], f32, tag="pu")
        nc.tensor.matmul(p_g2[:], bA("ag2psi"), _r(r2[:].rearrange("c h w -> c (h w)")))
        a2 = sbt.tile([1, 256], f32, tag="a2")
        nc.scalar.activation(_r(a2[:]), p_g2[:], AF.Sigmoid)
        p_a2b = pu.tile([128, 256], f32, tag="pu")
        nc.tensor.matmul(p_a2b[:], _r(bF("ones", 1)), _r(a2[:]))
        nc.vector.tensor_mul(_r(att2p[:, 1:17, 1:17]),
                             p_a2b[:].rearrange("c (h w) -> c h w", h=16, w=16), c2p[:])

        # ------------- dec2: 9 taps x 3 K-chunks ---------------------------
        pd2_cm = tc.tile_pool(name="pd2", bufs=1, space="PSUM")
        pd2p = pd2_cm.__enter__()
        p_d2 = pd2p.tile([128, 256], f32, tag="pd2")
        d2w = bB("dec2").rearrange("c (s t m) -> c s t m", s=3, t=9)
        srcs2 = (u2p0, u2p1, att2p)
        n_mm = 0
        for kc in range(3):
            for tap in range(9):
                dy, dx = tap // 3, tap % 3
                nc.tensor.matmul(p_d2[:], d2w[:, kc, tap, :],
                                 _r(srcs2[kc][:, dy:dy + 16, dx:dx + 16]),
                                 start=(n_mm == 0), stop=(n_mm == 26),
                                 skip_group_check=True)
                n_mm += 1
        nc.scalar.activation(_r(d2pad[:, 1:17, 1:17]),
                             p_d2[:].rearrange("c (h w) -> c h w", h=16, w=16),
                             AF.Relu, bias=bF("dec2b"))
        pd2_cm.__exit__(None, None, None)

        # ------------- attention gate 1 (pre-upsample trick) ---------------
        p_q1 = pu.tile([64, 16, 16], f32, tag="pu")
        nc.tensor.matmul(p_q1[:], bB("ag1wg"), _r(d2pad[:, 1:17, 1:17]))
        c1v = c1p[:].rearrange("c h w -> c (h w)")
        r1 = sbt.tile([64, 32, 32], f32, tag="r1")
        r1v = r1[:].rearrange("c h w -> c (h w)")
        a1 = sbt.tile([1, 1024], f32, tag="a1")
        r1ah = []
        for hh in range(2):
            p_x1 = pu.tile([64, 512], f32, tag="pu")
            nc.tensor.matmul(p_x1[:], bB("ag1wx", 64),
                             _r(c1v[:, hh * 512:(hh + 1) * 512]))
            r1a = sbt.tile([64, 512], f32, tag=f"r1a{hh}")
            q1b = (p_q1[:, 8 * hh:8 * hh + 8, :].unsqueeze(2).unsqueeze(4)
                   .broadcast_to([64, 8, 2, 16, 2]))
            nc.vector.tensor_tensor(
                out=r1a[:].rearrange("c (h a w b) -> c h a w b", h=8, a=2, w=16, b=2),
                in0=p_x1[:].rearrange("c (h a w b) -> c h a w b", h=8, a=2, w=16, b=2),
                in1=q1b, op=OP.add)
            nc.scalar.activation(r1v[:, hh * 512:(hh + 1) * 512], r1a[:], AF.Relu)
        for hh in range(2):
            p_g1 = pu.tile([1, 512], f32, tag="pu")
            nc.tensor.matmul(p_g1[:], bB("ag1psi", 64),
                             _r(r1v[:, hh * 512:(hh + 1) * 512]))
            nc.scalar.activation(_r(a1[:, hh * 512:(hh + 1) * 512]), p_g1[:], AF.Sigmoid)
            p_a1b = pu.tile([64, 512], f32, tag="pu")
            nc.tensor.matmul(p_a1b[:], _r(bF("ones", 1)[:, 0:64]),
                             _r(a1[:, hh * 512:(hh + 1) * 512]))
            nc.vector.tensor_mul(
                _r(att1p[:, 1 + 16 * hh:17 + 16 * hh, 1:33]),
                p_a1b[:].rearrange("c (h w) -> c h w", h=16, w=32),
                c1p[:, 16 * hh:16 * hh + 16, :])

        # premult tail (W1t stream lands late UNet)
        premult(w1t_s, 0)
        premult(w1t_s, 1)

        # ------------- dec1: 4-phase (u-part 2x2 cells + att 9 taps) -------
        # u-cells go into a dedicated psum pool right after d2pad is ready so
        # they fill the PE while the gate-1 chain runs.
        d1ph = bB("d1ph").rearrange("c (p l m) -> c p l m", p=4, l=4)
        d1at = bB("d1att", 64).rearrange("c (t m) -> c t m", t=9)
        pd1_cm = tc.tile_pool(name="pd1", bufs=1, space="PSUM")
        pd1 = pd1_cm.__enter__()
        p_d1a = pd1.tile([64, 2, 16, 16], f32, tag="pd1a")
        p_d1b = pd1.tile([64, 2, 16, 16], f32, tag="pd1b")
        p_d1t = [p_d1a, p_d1b]
        for a in range(2):
            for b in range(2):
                ph_i = a * 2 + b
                p_d1 = p_d1t[a][:, b, :, :]
                n_mm = 0
                for cu in range(2):
                    for cv in range(2):
                        nc.tensor.matmul(p_d1, d1ph[:, ph_i, cu * 2 + cv, :],
                                         _r(d2pad[:, cu + a:cu + a + 16,
                                                  cv + b:cv + b + 16]),
                                         start=(n_mm == 0), stop=False,
                                         skip_group_check=True)
                        n_mm += 1
        premult(w1t_s, 2)
        premult(w1t_s, 3)
        for a in range(2):
            for b in range(2):
                p_d1 = p_d1t[a][:, b, :, :]
                for tap in range(9):
                    dy, dx = tap // 3, tap % 3
                    nc.tensor.matmul(p_d1, d1at[:, tap, :],
                                     _r(att1p[:, a + dy:a + dy + 31:2,
                                              b + dx:b + dx + 31:2]),
                                     start=False, stop=(tap == 8),
                                     skip_group_check=True)
                nc.scalar.activation(_r(d1s[:, a:32:2, b:32:2]), p_d1,
                                     AF.Relu, bias=bF("dec1b", 64))
            if a == 0:
                premult(w1t_s, 4)
                premult(w1t_s, 5)

        nc.scalar.activation(et1[:], p_ew[:], AF.Copy)
        pd1_cm.__exit__(None, None, None)
        pw_cm.__exit__(None, None, None)

        # ------------- fin 1x1 conv -> amapT [256, 1024] -------------------
        d1v = d1s[:].rearrange("c h w -> c (h w)")
        c0, n = I16_COLS["pidx"]
        pidx = t_i16[:, c0:c0 + n]
        for mc, dst, htT in ((0, amap0, htT0), (1, amap1, htT1)):
            for hh in range(2):
                p_am = pu.tile([128, 512], f32, tag="pu")
                nc.tensor.matmul(p_am[:],
                                 bB("fin", 64)[:, mc * 128:(mc + 1) * 128],
                                 _r(d1v[:, hh * 512:(hh + 1) * 512]))
                nc.scalar.activation(dst[:, hh * 512:(hh + 1) * 512], p_am[:],
                                     AF.Identity, bias=bF("finb")[:, mc:mc + 1])
            nc.gpsimd.ap_gather(htT[:].rearrange("c (n o) -> c n o", o=1),
                                dst[:].rearrange("c (n o) -> c n o", o=1), pidx,
                                channels=128, num_elems=1024, d=1, num_idxs=NH)

        pu_cm.__exit__(None, None, None)

        # ------------- pair features + decoder -----------------------------
        ph_cm = tc.tile_pool(name="ph", bufs=2, space="PSUM")
        ph = ph_cm.__enter__()
        pd_cm = tc.tile_pool(name="pd", bufs=2, space="PSUM")
        pd = pd_cm.__enter__()
        po_cm = tc.tile_pool(name="po", bufs=1, space="PSUM")
        po = po_cm.__enter__()
        p_out = po.tile([2, NH], f32, tag="po")
        w2h = bC("w2h").rearrange("c (t m) -> c t m", t=2)
        w2t = bC("w2t").rearrange("c (t m) -> c t m", t=2)
        wde = bC("wdec").rearrange("c (g m) -> c g m", g=G)
        for k in range(KD):
            cols = slice(k * 128, (k + 1) * 128)
            for (w2, ewt, oh, bp, dstT) in ((w2h, ew1, ohhi, "hbp", hsT),
                                            (w2t, et1, ohti, "tbp", tsT)):
                p_hs = ph.tile([128, NH], f32, tag="ph")
                nc.tensor.matmul(p_hs[:], _r(ewt[:, cols]), _r(oh[:]),
                                 start=True, stop=False)
                nc.tensor.matmul(p_hs[:], w2[:, 0, cols], _r(htT0[:]),
                                 start=False, stop=False)
                nc.tensor.matmul(p_hs[:], w2[:, 1, cols], _r(htT1[:]),
                                 start=False, stop=True)
                nc.scalar.activation(dstT[:, k, :], p_hs[:],
                                     AF.Tanh, bias=bF(bp)[:, k:k + 1])
            for half in range(2):
                g = 2 * k + half
                rows = slice(half * 64, (half + 1) * 64)
                p_u = pd.tile([128, NH], f32, tag="pd")
                nc.tensor.matmul(p_u[:], wde[rows, g, :], tsT[rows, k, :])
                pu_sb = sbt.tile([128, NH], bf16, tag="pusb")
                if k < 4:
                    nc.gpsimd.tensor_copy(pu_sb[:], p_u[:])
                elif k == 4:
                    nc.scalar.activation(pu_sb[:], p_u[:], AF.Copy)
                else:
                    nc.vector.tensor_copy(pu_sb[:], p_u[:])
                v = sbt.tile([128, NH], bf16, tag="v")
                nc.vector.tensor_mul(v[0:64, :], pu_sb[0:64, :], hsT[rows, k, :])
                nc.vector.tensor_mul(v[64:128, :], pu_sb[64:128, :], hsT[rows, k, :])
                nc.tensor.matmul(p_out[:], bA("smat"), v[:],
                                 start=(g == 0), stop=(g == G - 1),
                                 skip_group_check=True)
        out_sb = sbt.tile([2, NH], f32, tag="out")
        nc.scalar.activation(out_sb[:], p_out[:], AF.Identity, bias=bF("decb", 2))
        nc.sync.dma_start(y[:], out_sb[:])
        po_cm.__exit__(None, None, None)
        pd_cm.__exit__(None, None, None)
        ph_cm.__exit__(None, None, None)

    nc.compile()
    return nc


def _wrap16(idx, n_slots):
    """int16 index layout for gpsimd gathers: wrapped in 16 partitions,
    replicated across the 8 gpsimd cores."""
    out = np.zeros((128, n_slots), np.int16)
    for j, v in enumerate(idx):
        out[np.arange(8) * 16 + j % 16, j // 16] = v
    return out


def pack_inputs(inputs):
    """Build the 8 per-core input maps from the full problem inputs."""
    import ml_dtypes
    x = np.asarray(inputs["x"], np.float32)
    entity_pos = np.asarray(inputs["entity_pos"])
    hts = np.asarray(inputs["hts"])

    def W(name):
        return np.asarray(inputs[name], np.float32)

    def blob(layout, ncols, parts_map):
        b = np.zeros((128, ncols), np.float32)
        for name, arr in parts_map.items():
            c0, n = layout[name]
            p = arr.shape[0]
            b[0:p, c0:c0 + n] = arr.reshape(p, n)
        return b

    shared = {}
    e1 = W("enc1_w").reshape(64, 9)            # [c, dy*3+dx]
    enc1 = np.zeros((3, 3 * 64), np.float32)   # [dx, dy*64+c]
    for dy in range(3):
        for dx in range(3):
            enc1[dx, dy * 64:(dy + 1) * 64] = e1[:, dy * 3 + dx]
    smat = np.zeros((128, 2), np.float32)
    smat[:64, 0] = 1.0
    smat[64:, 1] = 1.0
    blobA = blob(BA_COLS, CA, {
        "enc1": enc1,
        "enc2": W("enc2_w").reshape(128, 64, 9).transpose(1, 2, 0).copy(),
        "bott": W("bott_w").reshape(256, 128, 9).transpose(1, 2, 0).copy(),
        "ag2wg": W("ag2_wg").reshape(128, 256).T.reshape(2, 128, 128).transpose(1, 0, 2).copy(),
        "ag2wx": W("ag2_wx").reshape(128, 128).T.copy(),
        "ag2psi": W("ag2_psi").reshape(1, 128).T.copy(),
        "smat": smat,
    })
    d1w = W("dec1_w")                          # [64, 192, 3, 3]
    du = d1w[:, 0:128]                         # u-part [64, 128, 3, 3]
    d1ph = np.zeros((128, 4, 4, 64), np.float32)
    taps_u = {(0, 0): [0], (0, 1): [1, 2], (1, 0): [0, 1], (1, 1): [2]}
    for a in range(2):
        for b_ in range(2):
            for cu in range(2):
                for cv in range(2):
                    acc = np.zeros((128, 64), np.float32)
                    for dy in taps_u[(a, cu)]:
                        for dx in taps_u[(b_, cv)]:
                            acc += du[:, :, dy, dx].T
                    d1ph[:, a * 2 + b_, cu * 2 + cv, :] = acc
    blobB = blob(BB_COLS, CB, {
        "dec2": W("dec2_w").reshape(128, 384, 9).transpose(1, 2, 0)
                .reshape(3, 128, 9, 128).transpose(1, 0, 2, 3).copy(),
    })
    blobB2 = blob(BB2_COLS, CB2, {
        "ag1wg": W("ag1_wg").reshape(64, 128).T.copy(),
        "ag1wx": W("ag1_wx").reshape(64, 64).T.copy(),
        "ag1psi": W("ag1_psi").reshape(1, 64).T.copy(),
        "d1ph": d1ph,
        "d1att": d1w[:, 128:192].reshape(64, 64, 9).transpose(1, 2, 0).copy(),
        "fin": W("fin_w").reshape(256, 64).T.copy(),
    })
    head_w = W("head_w"); tail_w = W("tail_w")
    wd = W("decoder_w").reshape(G, 64, 64, 2).transpose(2, 0, 3, 1).reshape(64, G, 128)
    blobC = blob(BC_COLS, CC, {
        "w2h": head_w[D:].reshape(2, 128, D).transpose(1, 0, 2).copy(),
    })
    blobC2 = blob(BC2_COLS, CC2, {
        "w2t": tail_w[D:].reshape(2, 128, D).transpose(1, 0, 2).copy(),
        "wdec": np.concatenate([wd, wd], axis=0).copy(),
    })
    for k, v in (("blobA", blobA), ("blobB", blobB), ("blobB2", blobB2),
                 ("blobC", blobC), ("blobC2", blobC2)):
        shared[k] = v.astype(ml_dtypes.bfloat16)
    shared["W1h"] = head_w[:D].reshape(KD, 128, D).transpose(1, 0, 2).astype(ml_dtypes.bfloat16)
    shared["W1t"] = tail_w[:D].reshape(KD, 128, D).transpose(1, 0, 2).astype(ml_dtypes.bfloat16)

    f32_shared = {
        "ident": np.eye(NE, dtype=np.float32),
        "enc1b": W("enc1_b").reshape(64, 1),
        "enc2b": W("enc2_b").reshape(128, 1),
        "bottb": W("bott_b").reshape(2, 128).T.copy(),
        "dec2b": W("dec2_b").reshape(128, 1),
        "dec1b": W("dec1_b").reshape(64, 1),
        "finb": W("fin_b").reshape(2, 128).T.copy(),
        "hbp": W("head_b").reshape(KD, 128).T.copy(),
        "tbp": W("tail_b").reshape(KD, 128).T.copy(),
        "decb": W("decoder_b").reshape(2, 1),
        "iota": np.arange(NE, dtype=np.float32).reshape(NE, 1),
        "ones": np.ones((1, 128), np.float32),
    }

    in_maps = []
    for c in range(NCORES):
        b, h = c // 2, c % 2
        m = dict(shared)
        start = entity_pos[b, :, 0].astype(np.int64)
        idx = np.minimum(start + 1, L - 1)
        mask = (start + 1 < L).astype(np.float32).reshape(NE, 1)
        m["ent_in"] = np.ascontiguousarray(x[b][idx])
        fm = dict(f32_shared)
        fm["mask"] = mask
        f32bl = np.zeros((128, CF), np.float32)
        for name, arr in fm.items():
            c0, n = F32_COLS[name]
            p = arr.shape[0]
            f32bl[0:p, c0:c0 + n] = arr.reshape(p, n)
        m["f32b"] = f32bl
        hi = hts[b, h * NH:(h + 1) * NH, 0].astype(np.int64)
        ti = hts[b, h * NH:(h + 1) * NH, 1].astype(np.int64)
        fcb = np.zeros((1, CFC), np.float32)
        fcb[0, FC_COLS["hi_f"][0]:FC_COLS["hi_f"][0] + NH] = hi
        fcb[0, FC_COLS["ti_f"][0]:FC_COLS["ti_f"][0] + NH] = ti
        m["f32c"] = fcb
        i16bl = np.zeros((128, CI), np.int16)
        c0, n = I16_COLS["pidx"]
        i16bl[:, c0:c0 + n] = _wrap16((hi * NE + ti).astype(np.int16), NH // 16)
        m["i16b"] = i16bl
        in_maps.append(m)
    return in_maps


_NC_CACHE = None


def get_nc():
    global _NC_CACHE
    if _NC_CACHE is None:
        _NC_CACHE = build_nc()
    return _NC_CACHE


def kernel(**inputs):
    nc = get_nc()
    in_maps = pack_inputs(inputs)
    res = run_bass_kernel_spmd(nc, in_maps, core_ids=list(range(NCORES)))
    out = np.empty((B * P, 2), np.float32)
    for c in range(NCORES):
        b, h = c // 2, c % 2
        yc = res.results[c]["y"]                  # [2, NH]
        out[b * P + h * NH:b * P + (h + 1) * NH, :] = yc.T
    return out
